# revision 1
# baseline (speedup 1.0000x reference)
"""Multi-head attention (ReLU-gated projections) on 8 Trainium2 NeuronCores.

Problem (hardcoded): B=4, S=1024, H=1024, NH=16, DH=64.
  qp = relu(q @ Wq.T + bq); kp, vp likewise
  alpha = softmax(qh @ kh.T / sqrt(DH)) * mask[q]
  out = (alpha @ vh).reshape(B,S,H) + query

Sharding: 8 cores = 4 batches x 2 head-groups (8 heads / 512 hidden cols each).

Per-core device kernel (all in transposed "hidden-on-partitions" layout):
  stage 1: qpT[o,s], kpT[o,s] (transposed) and vp[s,o] (normal) projections
           with fused bias+relu. Contraction over h via PE; inputs fed
           host-pre-transposed (xT = x.T per batch).
  stage 2: per head: alphaT[k,q] = khT.T @ qhT (K=64); P=exp(alpha/8) on ACT
           (no max subtraction needed: alpha/8 <= ~5); AV via PE with a ones
           column appended to v so row 64 of the output accumulates
           sumexp[q] for free.  Output: unnormalized hidT (64,S) + sumexp (S)
           per head; host divides, applies mask, adds residual.
"""
import sys

sys.path.insert(0, "/opt/trn_rl_repo")

import os
import numpy as np
import ml_dtypes

import concourse.bass as bass
import concourse.tile as tile
from concourse import bacc, mybir
from concourse import bass_utils

if os.environ.get("BASS_LDW_OPT", "0") == "1":
    _orig_run_command = bass_utils.run_command

    def _patched_run_command(cmd, **kw):
        cmd = ["--enable-ldw-opt=true" if c == "--enable-ldw-opt=false" else c
               for c in cmd]
        return _orig_run_command(cmd, **kw)

    bass_utils.run_command = _patched_run_command

B, S, H = 4, 1024, 1024
NH, DH = 16, 64
NCORES = 8
GROUPS = 2          # head-groups (tensor-parallel dim)
HL = NH // GROUPS   # heads per core = 8
GH = H // GROUPS    # hidden cols per core = 512
KT = H // 128       # contraction k-tiles = 8
OT = GH // 128      # output o-tiles per core = 4
SCALE = 1.0 / float(np.sqrt(DH))

# matmul precision mode: "f32" (exact, 4 cyc/row), "f32r" (TF32-ish, 1 cyc/row),
# "bf16" (1 cyc/row, smallest footprint)
MODE = os.environ.get("BASS_MM_DT", "bf16")
ALPHA_ILV = os.environ.get("BASS_ALPHA_ILV", "1") == "1"

F32 = mybir.dt.float32
F32R = mybir.dt.float32r
BF16 = mybir.dt.bfloat16


def _cfg(mode):
    if mode == "bf16":
        return dict(np_dt=ml_dtypes.bfloat16, io_dt=BF16, st_dt=BF16,
                    cast=False, pt_bufs=36, hid_bufs=3, x_bufs=16,
                    shift_alphas=True, kz=True)
    if mode == "f32r":
        # float32r end-to-end: walrus requires f32r matmul inputs to be
        # *produced* as f32r (DMA loads + DVE/ACT evacuations), not bitcast.
        return dict(np_dt=np.float32, io_dt=F32R, st_dt=F32R,
                    cast=False, pt_bufs=9, hid_bufs=2, x_bufs=8,
                    shift_alphas=False, kz=False)
    return dict(np_dt=np.float32, io_dt=F32, st_dt=F32,
                cast=False, pt_bufs=9, hid_bufs=2, x_bufs=8,
                shift_alphas=False, kz=False)


def _mm(ap, cast):
    return ap.bitcast(F32R) if cast else ap


def build(mode):
    cfg = _cfg(mode)
    io_dt, st_dt, cast = cfg["io_dt"], cfg["st_dt"], cfg["cast"]
    nc = bacc.Bacc("TRN2", target_bir_lowering=False, debug=False,
                   num_devices=NCORES)

    xq_d = nc.dram_tensor("xq", [H, S], io_dt, kind="ExternalInput").ap()
    xk_d = nc.dram_tensor("xk", [H, S], io_dt, kind="ExternalInput").ap()
    xv_d = nc.dram_tensor("xv", [H, S], io_dt, kind="ExternalInput").ap()
    wq_d = nc.dram_tensor("wq", [H, GH], io_dt, kind="ExternalInput").ap()
    wk_d = nc.dram_tensor("wk", [H, GH], io_dt, kind="ExternalInput").ap()
    wv_d = nc.dram_tensor("wv", [H, GH], io_dt, kind="ExternalInput").ap()
    bqk_d = nc.dram_tensor("bqk", [128, 2 * OT], F32, kind="ExternalInput").ap()
    bv_d = nc.dram_tensor("bv", [1, GH], io_dt, kind="ExternalInput").ap()
    ones_d = nc.dram_tensor("onesd", [128, 128], io_dt,
                            kind="ExternalInput").ap()
    zeros_d = nc.dram_tensor("zerosd", [64, S], io_dt,
                             kind="ExternalInput").ap()
    hid_d = nc.dram_tensor("hid", [HL * (DH + 1), S], F32,
                           kind="ExternalOutput").ap()

    with tile.TileContext(nc) as tc:
        with tc.tile_pool(name="sb", bufs=1) as sb, \
             tc.tile_pool(name="ps", bufs=1, space="PSUM") as ps:

            full_x = mode == "bf16"   # x resident for full S vs per-chunk

            # ---- persistent tiles; one big DMA per tensor (>=1MB, descriptor
            #      runs of 1-2KB/partition), spread across the three DGE rings
            #      (sync / scalar / gpsimd) so loads overlap ----
            wq_t = sb.tile([128, KT * GH], io_dt, tag="wq", name="wq")
            wk_t = sb.tile([128, KT * GH], io_dt, tag="wk", name="wk")
            wv_t = sb.tile([128, KT * GH], io_dt, tag="wv", name="wv")
            qp_t = [sb.tile([128, S], st_dt, tag=f"qp{t}", name=f"qp{t}")
                    for t in range(OT)]
            KZ = cfg["kz"]
            if KZ:
                # zero-padded K copies: kz[t][h] holds head h's kh rows in its
                # own 64-partition half, zeros in the other -> full-K=128
                # alpha matmuls whose weight loads pipeline like any other MM
                kz_t = [[sb.tile([128, S], st_dt, tag=f"kz{t}{h}",
                                 name=f"kz{t}{h}") for h in range(2)]
                        for t in range(OT)]
                kz_zeroed = set()
            else:
                kp_t = [sb.tile([128, S], st_dt, tag=f"kp{t}",
                                name=f"kp{t}") for t in range(OT)]
            # v laid out [k-tile x head x (64 v cols + ones col)]
            VW = HL * (DH + 1)
            vp_t = sb.tile([128, KT * VW], st_dt, tag="vp", name="vp")
            bqk_t = sb.tile([128, 2 * OT], F32, tag="bqk", name="bqk")
            bv_t = sb.tile([1, GH], io_dt, tag="bv", name="bv")
            ones_t = sb.tile([1, 128], io_dt, tag="ones", name="ones")

            def w_ld(w_t, w_d, eng):
                # SBUF [128, k*GH + o]  <-  DRAM [(k p) o]
                eng.dma_start(w_t[:].rearrange("p (k o) -> p k o", o=GH),
                              w_d.rearrange("(k p) o -> p k o", p=128))

            x_t = {}

            def x_ld(which, x_d, eng, sc, eng2=None):
                # one chunk (512 s-cols) of one input, laid [128, k*512+s];
                # optionally split across two DGE rings for latency
                tag = f"x{which}{sc}" if full_x else f"x{which}"
                t = sb.tile([128, KT * 512], io_dt, tag=tag,
                            name=f"x{which}_{sc}")
                src3 = x_d.rearrange("(k p) s -> p k s",
                                     p=128)[:, :, sc * 512:(sc + 1) * 512]
                dst3 = t[:].rearrange("p (k s) -> p k s", s=512)
                if eng2 is None:
                    eng.dma_start(dst3, src3)
                else:
                    h = KT // 2
                    eng.dma_start(dst3[:, :h], src3[:, :h])
                    eng2.dma_start(dst3[:, h:], src3[:, h:])
                x_t[(which, sc)] = t

            def xap(which, sc, k):
                return x_t[(which, sc)][:, k * 512:(k + 1) * 512]

            nc.sync.dma_start(bv_t[:], bv_d)
            nc.sync.dma_start(ones_t[:], ones_d[0:1, :])
            nc.sync.dma_start(bqk_t[:], bqk_d)
            ones64_t = sb.tile([128, KT * HL], io_dt, tag="ones64",
                               name="ones64")
            nc.sync.dma_start(ones64_t[:], ones_d[:, 0:KT * HL])
            x_ld("q", xq_d, nc.scalar, 0)
            w_ld(wq_t, wq_d, nc.sync)
            x_ld("k", xk_d, nc.sync, 0)
            w_ld(wk_t, wk_d, nc.scalar)
            x_ld("v", xv_d, nc.scalar, 0)
            w_ld(wv_t, wv_d, nc.sync)

            # HAM warmup: ~8us of tiny matmuls on early-arriving const tiles
            # so the real matmuls start at 2.4GHz instead of 1.2
            warm = ps.tile([128, 512], F32, tag="av", bufs=2, name="warm")
            for i in range(22):
                nc.tensor.matmul(warm[:], _mm(ones_t[:], cast),
                                 _mm(bv_t[:], cast), start=True, stop=True)
            # preload the ACT exp table set (~2.7us) while ACT is still idle
            dummy_exp = sb.tile([1, 8], F32, tag="dummy_exp", name="dummy_exp")
            nc.scalar.activation(dummy_exp[:], ones_t[0:1, 0:8],
                                 mybir.ActivationFunctionType.Exp, scale=1.0)
            if full_x:
                x_ld("q", xq_d, nc.sync, 1)
                x_ld("k", xk_d, nc.sync, 1)
                x_ld("v", xv_d, nc.gpsimd, 1)
            v4 = vp_t[:].rearrange("p (k n c) -> p k n c", n=HL, c=DH + 1)
            nc.vector.tensor_copy(
                v4[:, :, :, DH:DH + 1],
                ones64_t[:].rearrange("p (k n one) -> p k n one", n=HL, one=1))

            def proj_qk(sc, ot, only=None):
                """one o-tile, one s-chunk of the transposed q/k projections"""
                for which, w_t, xw in (("q", wq_t, "q"), ("k", wk_t, "k")):
                    if only is not None and which != only:
                        continue
                    pp = ps.tile([128, 1024], F32, tag="alpha", bufs=3,
                                 name=f"pp{which}_{sc}_{ot}")
                    for k in range(KT):
                        nc.tensor.matmul(
                            pp[:, 0:512],
                            _mm(w_t[:, k * GH + ot * 128:
                                    k * GH + (ot + 1) * 128], cast),
                            _mm(xap(xw, sc, k), cast),
                            start=(k == 0), stop=(k == KT - 1))
                    wi = 0 if which == "q" else 1
                    bias = bqk_t[:, wi * OT + ot:wi * OT + ot + 1]
                    ssl = slice(sc * 512, (sc + 1) * 512)

                    def evac(dst, pslice, bias_ap):
                        # chunk-0 evacs go to the (idle-at-that-point) ACT
                        # engine: relu(x*1 + bias); chunk-1 to DVE
                        if sc == 0 and KZ:
                            nc.scalar.activation(
                                dst, pslice,
                                mybir.ActivationFunctionType.Relu,
                                bias=bias_ap, scale=1.0)
                        else:
                            nc.vector.tensor_scalar(
                                dst, pslice, bias_ap, 0.0,
                                mybir.AluOpType.add, mybir.AluOpType.max)

                    if which == "q":
                        evac(qp_t[ot][:, ssl], pp[:, 0:512], bias)
                    elif KZ:
                        for h in range(2):
                            pr = slice(h * 64, h * 64 + 64)
                            evac(kz_t[ot][h][pr, ssl], pp[pr, 0:512],
                                 bias[pr, :])
                    else:
                        nc.vector.tensor_scalar(
                            kp_t[ot][:, ssl], pp[:, 0:512], bias, 0.0,
                            mybir.AluOpType.add, mybir.AluOpType.max)

            def proj_v(sc, j):
                """one s-tile (128 rows of vp) within chunk sc"""
                st = sc * 4 + j
                pp = ps.tile([128, 1024], F32, tag="alpha", bufs=3,
                             name=f"ppv_{st}")
                nc.tensor.matmul(pp[:, 0:512], _mm(ones_t[:], cast),
                                 _mm(bv_t[:], cast), start=True, stop=False)
                for k in range(KT):
                    nc.tensor.matmul(
                        pp[:, 0:512],
                        _mm(xap("v", sc, k)[:, j * 128:(j + 1) * 128], cast),
                        _mm(wv_t[:, k * GH:(k + 1) * GH], cast),
                        start=False, stop=(k == KT - 1))
                v3 = vp_t[:, st * VW:(st + 1) * VW].rearrange(
                    "p (n c) -> p n c", c=DH + 1)
                p3 = pp[:, 0:512].rearrange("p (n c) -> p n c", c=DH)
                nc.vector.tensor_scalar(
                    v3[:, :, 0:DH], p3, 0.0, None, mybir.AluOpType.max)

            pt_all = {}

            def alphas(n0):
                """alpha + exp for head pair (n0, n0+1); the two heads live on
                disjoint 64-partition halves of o-tile n0//2, so adjacent
                matmuls target disjoint PE row-groups and overlap."""
                t = n0 // 2
                if t not in kz_zeroed:
                    kz_zeroed.add(t)
                    nc.vector.memset(kz_t[t][0][64:128, :], 0.0)
                    nc.vector.memset(kz_t[t][1][0:64, :], 0.0)
                pts0, pts1 = [], []
                for k in range(KT):
                    apts = []
                    for h in range(2):
                        apt = ps.tile([128, 1024], F32, tag="alpha", bufs=3,
                                      name=f"alp_{n0 + h}_{k}")
                        apts.append(apt)
                    for qc in range(2):
                        for h in range(2):
                            nc.tensor.matmul(
                                apts[h][:, qc * 512:(qc + 1) * 512],
                                _mm(kz_t[t][h][:, k * 128:(k + 1) * 128],
                                    cast),
                                _mm(qp_t[t][:, qc * 512:(qc + 1) * 512],
                                    cast),
                                start=True, stop=True)
                    for h, pts in ((0, pts0), (1, pts1)):
                        pt = sb.tile([128, 1024], st_dt, tag="pt",
                                     bufs=cfg["pt_bufs"], name=f"pt_{n0 + h}_{k}")
                        nc.scalar.activation(pt[:], apts[h][:],
                                             mybir.ActivationFunctionType.Exp,
                                             scale=SCALE)
                        pts.append(pt)
                pt_all[n0] = pts0
                pt_all[n0 + 1] = pts1

            def head_seq(n):
                """unpaired alpha+exp then AV for one head (low pt_bufs modes)"""
                t, off = n // 2, (n % 2) * 64
                pts = []
                for k in range(KT):
                    apt = ps.tile([128, 1024], F32, tag="alpha", bufs=3,
                                  name=f"alp_{n}_{k}")
                    for qc in range(2):
                        nc.tensor.matmul(
                            apt[:, qc * 512:(qc + 1) * 512],
                            _mm(kp_t[t][off:off + 64,
                                        k * 128:(k + 1) * 128], cast),
                            _mm(qp_t[t][off:off + 64,
                                        qc * 512:(qc + 1) * 512], cast),
                            start=True, stop=True)
                    pt = sb.tile([128, 1024], st_dt, tag="pt",
                                 bufs=cfg["pt_bufs"], name=f"pt_{n}_{k}")
                    nc.scalar.activation(pt[:], apt[:],
                                         mybir.ActivationFunctionType.Exp,
                                         scale=SCALE)
                    pts.append(pt)
                pt_all[n] = pts

            def avs(n):
                pts = pt_all.pop(n)
                hid_t = sb.tile([DH + 1, S], F32, tag="hid",
                                bufs=cfg["hid_bufs"], name=f"hid_{n}")
                for qc in range(2):
                    av = ps.tile([DH + 1, 512], F32, tag="av", bufs=2,
                                 name=f"av_{n}_{qc}")
                    for k in range(KT):
                        nc.tensor.matmul(
                            av[:],
                            _mm(vp_t[:, k * VW + n * (DH + 1):
                                     k * VW + (n + 1) * (DH + 1)], cast),
                            _mm(pts[k][:, qc * 512:(qc + 1) * 512], cast),
                            start=(k == 0), stop=(k == KT - 1))
                    nc.vector.tensor_copy(
                        hid_t[:, qc * 512:(qc + 1) * 512], av[:])
                    nc.sync.dma_start(
                        hid_d[n * (DH + 1):(n + 1) * (DH + 1),
                              qc * 512:(qc + 1) * 512],
                        hid_t[:, qc * 512:(qc + 1) * 512])

            # ---- emission schedule ----
            if cfg["shift_alphas"]:
                for ot in range(OT):
                    proj_qk(0, ot, only="q")
                for ot in range(OT):
                    proj_qk(0, ot, only="k")
            else:
                for ot in range(OT):
                    proj_qk(0, ot)
            for j in range(4):
                proj_v(0, j)
            if not full_x:
                x_ld("q", xq_d, nc.sync, 1)
                x_ld("k", xk_d, nc.sync, 1)
                x_ld("v", xv_d, nc.gpsimd, 1)
            if cfg["shift_alphas"]:
                proj_qk(1, 0)
                alphas(0)
                for j in range(4):
                    proj_v(1, j)
                proj_qk(1, 1)
                alphas(2)
                avs(0)
                avs(1)
                proj_qk(1, 2)
                alphas(4)
                avs(2)
                avs(3)
                proj_qk(1, 3)
                alphas(6)
                avs(4)
                avs(5)
                avs(6)
                avs(7)
            else:
                proj_qk(1, 0)
                head_seq(0)
                for j in range(4):
                    proj_v(1, j)
                head_seq(1)
                avs(0)
                avs(1)
                for ot in range(1, OT):
                    proj_qk(1, ot)
                    head_seq(2 * ot)
                    avs(2 * ot)
                    head_seq(2 * ot + 1)
                    avs(2 * ot + 1)

    nc.compile()
    return nc


_NC_CACHE = {}


def _get_nc(mode):
    if mode not in _NC_CACHE:
        _NC_CACHE[mode] = build(mode)
    return _NC_CACHE[mode]


def _prep_inputs(inputs, mode):
    cfg = _cfg(mode)
    np_dt = cfg["np_dt"]
    q = np.asarray(inputs["query"], np.float32)
    k = np.asarray(inputs["key"], np.float32)
    v = np.asarray(inputs["value"], np.float32)
    Wq = np.asarray(inputs["Wq"], np.float32)
    Wk = np.asarray(inputs["Wk"], np.float32)
    Wv = np.asarray(inputs["Wv"], np.float32)
    bq = np.asarray(inputs["bq"], np.float32)
    bk = np.asarray(inputs["bk"], np.float32)
    bv = np.asarray(inputs["bv"], np.float32)

    xq = [np.ascontiguousarray(q[b].T).astype(np_dt) for b in range(B)]
    xk = [np.ascontiguousarray(k[b].T).astype(np_dt) for b in range(B)]
    xv = [np.ascontiguousarray(v[b].T).astype(np_dt) for b in range(B)]
    in_maps = []
    for c in range(NCORES):
        b, g = c // GROUPS, c % GROUPS
        sl = slice(g * GH, (g + 1) * GH)
        bqk = np.stack([bq[sl].reshape(OT, 128).T,
                        bk[sl].reshape(OT, 128).T], 1).reshape(128, 2 * OT)
        in_maps.append({
            "xq": xq[b], "xk": xk[b], "xv": xv[b],
            "wq": np.ascontiguousarray(Wq[sl, :].T).astype(np_dt),
            "wk": np.ascontiguousarray(Wk[sl, :].T).astype(np_dt),
            "wv": np.ascontiguousarray(Wv[sl, :].T).astype(np_dt),
            "bqk": np.ascontiguousarray(bqk, dtype=np.float32),
            "bv": np.ascontiguousarray(bv[None, sl]).astype(np_dt),
            "onesd": np.ones((128, 128), np_dt),
            "zerosd": np.zeros((64, S), np_dt),
        })
    return in_maps


def run(inputs, mode=MODE, trace=False):
    nc = _get_nc(mode)
    in_maps = _prep_inputs(inputs, mode)
    res = bass_utils.run_bass_kernel_spmd(
        nc, in_maps, core_ids=list(range(NCORES)), trace=trace)

    masks = np.asarray(inputs["masks"], np.float32)
    query = np.asarray(inputs["query"], np.float32)
    out = np.empty((B, S, H), np.float32)
    for c in range(NCORES):
        b, g = c // GROUPS, c % GROUPS
        hid = res.results[c]["hid"].reshape(HL, DH + 1, S)
        hT = hid[:, :DH, :]                      # (HL, DH, S)
        se = hid[:, DH, :]                       # (HL, S)
        blk = (hT / se[:, None, :]).transpose(2, 0, 1).reshape(S, GH)
        out[b, :, g * GH:(g + 1) * GH] = blk
    out = out * masks[:, :, None] + query
    return out, res


def kernel(**inputs) -> np.ndarray:
    out, _ = run(inputs)
    return out



# revision 12
# speedup vs baseline: 1.3317x; 1.3317x over previous
"""Multi-head attention (ReLU-gated projections) on 8 Trainium2 NeuronCores.

Problem (hardcoded): B=4, S=1024, H=1024, NH=16, DH=64.
  qp = relu(q @ Wq.T + bq); kp, vp likewise
  alpha = softmax(qh @ kh.T / sqrt(DH)) * mask[q]
  out = (alpha @ vh).reshape(B,S,H) + query

Sharding: 8 cores = 4 batches x 2 head-groups (8 heads / 512 hidden cols each).

fp8 mode (default): all matmuls in float8_e4m3. Weights are pre-scaled by
64 on the host so their N(0, 1/32) values land in e4m3's normal range;
the 64x factors ride through the linear pipeline (qp,kp,vp all carry 64x)
and are compensated in the exp scale (1/(8*64^2)) and a final /64 on the
host. Projections and AV use MatmulPerfMode.DoubleRow (K=256 per
instruction, 2x PE throughput); alpha matmuls are output-rate-bound so
they stay in plain fp8 with the kz zero-padded-K trick. The AV stationary
keeps a ones column (M=65) so row 64 accumulates sumexp for free; the
per-head V slot is padded to 68 bytes so every DoubleRow weight AP is
4-byte aligned (ISA restriction s3_lw_dual_fp8).

Per-core device kernel (transposed "hidden-on-partitions" layout):
  stage 1: qpT[o,s], kpT[o,s] (transposed) and vp[s,o] (normal) projections
           with fused bias+relu, evacuated to fp8.
  stage 2: per head pair: alphaT[k,q] psum tiles; pt = exp(alpha/8/4096) in
           fp8 written into paired [128, 2048] tiles; AV via DoubleRow with
           ones column -> unnormalized hidT (64,S) + sumexp (S) per head;
           host divides, applies mask, adds residual.
"""
import sys

sys.path.insert(0, "/opt/trn_rl_repo")

import os
import numpy as np
import ml_dtypes

import concourse.bass as bass
import concourse.tile as tile
from concourse import bacc, mybir
from concourse import bass_utils

B, S, H = 4, 1024, 1024
NH, DH = 16, 64
NCORES = 8
GROUPS = 2          # head-groups (tensor-parallel dim)
HL = NH // GROUPS   # heads per core = 8
GH = H // GROUPS    # hidden cols per core = 512
KT = H // 128       # contraction k-tiles = 8
OT = GH // 128      # output o-tiles per core = 4
SCALE = 1.0 / float(np.sqrt(DH))
ESC = 32.0          # fp8 weight pre-scale (TRN2 fp8e4 = e4m3-with-inf,
                    # max finite 240: keep relu'd projections under ~170)
VW8 = HL * 68       # padded per-head v slot (64 v + 1 ones + 3 pad) = 544

MODE = os.environ.get("BASS_MM_DT", "fp8")

F32 = mybir.dt.float32
BF16 = mybir.dt.bfloat16
FP8 = mybir.dt.float8e4
DR = mybir.MatmulPerfMode.DoubleRow
E4 = ml_dtypes.float8_e4m3   # e4m3 WITH inf (max 240) — matches TRN2 hw


def build_fp8():
    nc = bacc.Bacc("TRN2", target_bir_lowering=False, debug=False,
                   num_devices=NCORES)

    xq_d = nc.dram_tensor("xq", [H, S], FP8, kind="ExternalInput").ap()
    xk_d = nc.dram_tensor("xk", [H, S], FP8, kind="ExternalInput").ap()
    xv_d = nc.dram_tensor("xv", [H, S], FP8, kind="ExternalInput").ap()
    wq_d = nc.dram_tensor("wq", [H, GH], FP8, kind="ExternalInput").ap()
    wk_d = nc.dram_tensor("wk", [H, GH], FP8, kind="ExternalInput").ap()
    wv_d = nc.dram_tensor("wv", [H, GH], FP8, kind="ExternalInput").ap()
    bqk_d = nc.dram_tensor("bqk", [128, 2 * OT], F32, kind="ExternalInput").ap()
    bv_d = nc.dram_tensor("bv", [1, GH], FP8, kind="ExternalInput").ap()
    ones_d = nc.dram_tensor("onesd", [128, 128], FP8,
                            kind="ExternalInput").ap()
    hid_d = nc.dram_tensor("hid", [HL * (DH + 1), S], F32,
                           kind="ExternalOutput").ap()
    DEBUG = os.environ.get("BASS_FP8_DEBUG", "0") == "1"
    if DEBUG:
        dbg_qp_d = nc.dram_tensor("dbg_qp", [128, S], FP8,
                                  kind="ExternalOutput").ap()
        dbg_kz_d = nc.dram_tensor("dbg_kz", [128, S], FP8,
                                  kind="ExternalOutput").ap()
        dbg_al_d = nc.dram_tensor("dbg_al", [128, 1024], F32,
                                  kind="ExternalOutput").ap()
        dbg_pt_d = nc.dram_tensor("dbg_pt", [128, 2048], FP8,
                                  kind="ExternalOutput").ap()
        dbg_vp_d = nc.dram_tensor("dbg_vp", [128, KT * VW8], FP8,
                                  kind="ExternalOutput").ap()

    EXP_SCALE = SCALE / (ESC * ESC)
    EXP_BIAS = -3.0   # pt = exp(alpha/8 - 3): keeps exp under e4m3 max 240;
                      # cancels in hid/sumexp

    with tile.TileContext(nc) as tc:
        with tc.tile_pool(name="sb", bufs=1) as sb, \
             tc.tile_pool(name="ps", bufs=1, space="PSUM") as ps:

            # ---- persistent tiles ----
            wq_t = sb.tile([128, KT * GH], FP8, tag="wq", name="wq")
            wk_t = sb.tile([128, KT * GH], FP8, tag="wk", name="wk")
            wv_t = sb.tile([128, KT * GH], FP8, tag="wv", name="wv")
            qp_t = [sb.tile([128, S], FP8, tag=f"qp{t}", name=f"qp{t}")
                    for t in range(OT)]
            # zero-padded K copies for alpha (kz trick): head h of o-tile t
            # lives in its own 64-partition half, other half zeroed
            kz_t = [[sb.tile([128, S], FP8, tag=f"kz{t}{h}",
                             name=f"kz{t}{h}") for h in range(2)]
                    for t in range(OT)]
            kz_zeroed = set()
            # v laid out [s-tile x head x (64 v + 1 ones + 3 pad)]
            vp_t = sb.tile([128, KT * VW8], FP8, tag="vp", name="vp")
            bqk_t = sb.tile([128, 2 * OT], F32, tag="bqk", name="bqk")
            bv_t = sb.tile([1, GH], FP8, tag="bv", name="bv")
            ones_t = sb.tile([1, 128], FP8, tag="ones", name="ones")
            ones64_t = sb.tile([128, KT * HL], FP8, tag="ones64",
                               name="ones64")
            expb_t = sb.tile([128, 1], F32, tag="expb", name="expb")
            nc.vector.memset(expb_t[:], EXP_BIAS)

            def w_ld(w_t, w_d, eng):
                eng.dma_start(w_t[:].rearrange("p (k o) -> p k o", o=GH),
                              w_d.rearrange("(k p) o -> p k o", p=128))

            x_t = {}

            def x_ld(which, x_d, eng, sc, eng2=None):
                tag = f"x{which}{sc}"
                t = sb.tile([128, KT * 512], FP8, tag=tag,
                            name=f"x{which}_{sc}")
                src3 = x_d.rearrange("(k p) s -> p k s",
                                     p=128)[:, :, sc * 512:(sc + 1) * 512]
                dst3 = t[:].rearrange("p (k s) -> p k s", s=512)
                if eng2 is None:
                    eng.dma_start(dst3, src3)
                else:
                    h = KT // 2
                    eng.dma_start(dst3[:, :h], src3[:, :h])
                    eng2.dma_start(dst3[:, h:], src3[:, h:])
                x_t[(which, sc)] = t

            def x3(which, sc):
                return x_t[(which, sc)][:].rearrange("p (k s) -> p k s", s=512)

            # q/k both chunks first (first alphas gate on them), then weights,
            # then v
            x_ld("q", xq_d, nc.scalar, 0)
            x_ld("k", xk_d, nc.sync, 0)
            x_ld("q", xq_d, nc.gpsimd, 1)
            w_ld(wq_t, wq_d, nc.sync)
            x_ld("k", xk_d, nc.scalar, 1)
            w_ld(wk_t, wk_d, nc.sync)
            nc.scalar.dma_start(bqk_t[:], bqk_d)
            nc.gpsimd.dma_start(bv_t[:], bv_d)
            nc.gpsimd.dma_start(ones_t[:], ones_d[0:1, :])
            nc.gpsimd.dma_start(ones64_t[:], ones_d[:, 0:KT * HL])
            w_ld(wv_t, wv_d, nc.gpsimd)
            x_ld("v", xv_d, nc.sync, 0)
            x_ld("v", xv_d, nc.scalar, 1)

            # HAM warmup: tiny matmuls on early-arriving const tiles ramp the
            # PE to 2.4GHz while DMAs land
            warm = ps.tile([65, 512], F32, tag="av", bufs=2, name="warm")
            for i in range(22):
                nc.tensor.matmul(warm[:], ones_t[:, 0:65], bv_t[:],
                                 start=True, stop=True)
            # preload the ACT exp table while ACT is idle
            dummy_exp = sb.tile([1, 8], F32, tag="dummy_exp", name="dummy_exp")
            nc.scalar.activation(dummy_exp[:], ones_t[0:1, 0:8],
                                 mybir.ActivationFunctionType.Exp, scale=1.0)

            # ones column of the AV stationary
            v4 = vp_t[:].rearrange("p (k n c) -> p k n c", n=HL, c=68)
            nc.vector.tensor_copy(
                v4[:, :, :, DH:DH + 1],
                ones64_t[:].rearrange("p (k n one) -> p k n one", n=HL, one=1))

            def proj_qk(sc, ot, which):
                """one o-tile, one s-chunk of the transposed q/k projection"""
                w_t = wq_t if which == "q" else wk_t
                pp = ps.tile([128, 1024], F32, tag="alpha", bufs=3,
                             name=f"pp{which}_{sc}_{ot}")
                w3 = w_t[:].rearrange("p (k o) -> p k o", o=GH)
                xv_ = x3(which, sc)
                for kp in range(KT // 2):
                    nc.tensor.matmul(
                        pp[:, 0:512],
                        w3[:, 2 * kp:2 * kp + 2, ot * 128:(ot + 1) * 128],
                        xv_[:, 2 * kp:2 * kp + 2, :],
                        start=(kp == 0), stop=(kp == KT // 2 - 1),
                        perf_mode=DR)
                wi = 0 if which == "q" else 1
                bias = bqk_t[:, wi * OT + ot:wi * OT + ot + 1]
                ssl = slice(sc * 512, (sc + 1) * 512)
                if which == "q":
                    nc.vector.tensor_scalar(
                        qp_t[ot][:, ssl], pp[:, 0:512], bias, 0.0,
                        mybir.AluOpType.add, mybir.AluOpType.max)
                else:
                    for h in range(2):
                        pr = slice(h * 64, h * 64 + 64)
                        nc.vector.tensor_scalar(
                            kz_t[ot][h][pr, ssl], pp[pr, 0:512], bias[pr, :],
                            0.0, mybir.AluOpType.add, mybir.AluOpType.max)

            def proj_v(sc, j):
                """one s-tile (128 rows of vp) within chunk sc"""
                st = sc * 4 + j
                pp = ps.tile([128, 1024], F32, tag="alpha", bufs=3,
                             name=f"ppv_{st}")
                nc.tensor.matmul(pp[:, 0:512], ones_t[:], bv_t[:],
                                 start=True, stop=False)
                wv3 = wv_t[:].rearrange("p (k o) -> p k o", o=GH)
                xv_ = x3("v", sc)
                for kp in range(KT // 2):
                    nc.tensor.matmul(
                        pp[:, 0:512],
                        xv_[:, 2 * kp:2 * kp + 2, j * 128:(j + 1) * 128],
                        wv3[:, 2 * kp:2 * kp + 2, :],
                        start=False, stop=(kp == KT // 2 - 1),
                        perf_mode=DR)
                v3 = vp_t[:, st * VW8:(st + 1) * VW8].rearrange(
                    "p (n c) -> p n c", c=68)
                p3 = pp[:, 0:512].rearrange("p (n c) -> p n c", c=DH)
                nc.vector.tensor_scalar(
                    v3[:, :, 0:DH], p3, 0.0, None, mybir.AluOpType.max)

            pt_all = {}

            def alphas(n0):
                """alpha + exp for head pair (n0, n0+1); the two heads live on
                disjoint 64-partition halves of o-tile n0//2, so adjacent
                matmuls target disjoint PE row-groups and overlap.  pt tiles
                are paired [128, 2048] so AV can consume k-tile pairs via
                DoubleRow."""
                t = n0 // 2
                if t not in kz_zeroed:
                    kz_zeroed.add(t)
                    nc.gpsimd.memset(kz_t[t][0][64:128, :], 0.0)
                    nc.gpsimd.memset(kz_t[t][1][0:64, :], 0.0)
                pts0, pts1 = [], []
                cur = [None, None]
                for k in range(KT):
                    apts = []
                    for h in range(2):
                        apt = ps.tile([128, 1024], F32, tag="alpha", bufs=3,
                                      name=f"alp_{n0 + h}_{k}")
                        apts.append(apt)
                    for qc in range(2):
                        for h in range(2):
                            nc.tensor.matmul(
                                apts[h][:, qc * 512:(qc + 1) * 512],
                                kz_t[t][h][:, k * 128:(k + 1) * 128],
                                qp_t[t][:, qc * 512:(qc + 1) * 512],
                                start=True, stop=True)
                    half = k % 2
                    for h, pts in ((0, pts0), (1, pts1)):
                        if half == 0:
                            cur[h] = sb.tile([128, 2048], FP8, tag="pt",
                                             bufs=16, name=f"pt_{n0 + h}_{k}")
                            pts.append(cur[h])
                        nc.scalar.activation(
                            cur[h][:, half * 1024:(half + 1) * 1024],
                            apts[h][:],
                            mybir.ActivationFunctionType.Exp, scale=EXP_SCALE,
                            bias=expb_t[:])
                        if DEBUG and n0 == 0 and h == 0 and k == 0:
                            dbg_al_t = sb.tile([128, 1024], F32, tag="dbgal",
                                               name="dbgal")
                            nc.vector.tensor_copy(dbg_al_t[:], apts[h][:])
                            nc.sync.dma_start(dbg_al_d, dbg_al_t[:])
                        if DEBUG and n0 == 0 and h == 0 and k == 1:
                            nc.sync.dma_start(dbg_pt_d, cur[h][:])
                pt_all[n0] = pts0
                pt_all[n0 + 1] = pts1

            def avs(n):
                pts = pt_all.pop(n)
                hid_t = sb.tile([DH + 1, S], F32, tag="hid",
                                bufs=3, name=f"hid_{n}")
                for qc in range(2):
                    av = ps.tile([DH + 1, 512], F32, tag="av", bufs=2,
                                 name=f"av_{n}_{qc}")
                    for kp in range(KT // 2):
                        nc.tensor.matmul(
                            av[:],
                            v4[:, 2 * kp:2 * kp + 2, n, 0:DH + 1],
                            pts[kp][:].rearrange(
                                "p (k s) -> p k s",
                                s=1024)[:, :, qc * 512:(qc + 1) * 512],
                            start=(kp == 0), stop=(kp == KT // 2 - 1),
                            perf_mode=DR)
                    nc.vector.tensor_copy(
                        hid_t[:, qc * 512:(qc + 1) * 512], av[:])
                    nc.sync.dma_start(
                        hid_d[n * (DH + 1):(n + 1) * (DH + 1),
                              qc * 512:(qc + 1) * 512],
                        hid_t[:, qc * 512:(qc + 1) * 512])

            # ---- emission schedule: get head-pair 0's alphas going ASAP so
            #      the ACT exp stream (the co-bottleneck) starts early ----
            proj_qk(0, 0, "q")
            proj_qk(0, 0, "k")
            proj_qk(1, 0, "q")
            proj_qk(1, 0, "k")
            alphas(0)
            if DEBUG:
                nc.sync.dma_start(dbg_qp_d, qp_t[0][:])
                nc.sync.dma_start(dbg_kz_d, kz_t[0][0][:])
            proj_qk(0, 1, "q")
            proj_qk(0, 1, "k")
            proj_qk(1, 1, "q")
            proj_qk(1, 1, "k")
            for j in range(4):
                proj_v(0, j)
            for j in range(4):
                proj_v(1, j)
            if DEBUG:
                nc.sync.dma_start(dbg_vp_d, vp_t[:])
            alphas(2)
            avs(0)
            avs(1)
            proj_qk(0, 2, "q")
            proj_qk(0, 2, "k")
            proj_qk(1, 2, "q")
            proj_qk(1, 2, "k")
            alphas(4)
            avs(2)
            avs(3)
            proj_qk(0, 3, "q")
            proj_qk(0, 3, "k")
            proj_qk(1, 3, "q")
            proj_qk(1, 3, "k")
            alphas(6)
            avs(4)
            avs(5)
            avs(6)
            avs(7)

    nc.compile()
    return nc


_NC_CACHE = {}


def _get_nc(mode):
    if mode not in _NC_CACHE:
        if mode != "fp8":
            raise ValueError(f"unsupported mode {mode}")
        _NC_CACHE[mode] = build_fp8()
    return _NC_CACHE[mode]


def _prep_inputs(inputs):
    q = np.asarray(inputs["query"], np.float32)
    k = np.asarray(inputs["key"], np.float32)
    v = np.asarray(inputs["value"], np.float32)
    Wq = np.asarray(inputs["Wq"], np.float32)
    Wk = np.asarray(inputs["Wk"], np.float32)
    Wv = np.asarray(inputs["Wv"], np.float32)
    bq = np.asarray(inputs["bq"], np.float32)
    bk = np.asarray(inputs["bk"], np.float32)
    bv = np.asarray(inputs["bv"], np.float32)

    xq = [np.ascontiguousarray(q[b].T).astype(E4) for b in range(B)]
    xk = [np.ascontiguousarray(k[b].T).astype(E4) for b in range(B)]
    xv = [np.ascontiguousarray(v[b].T).astype(E4) for b in range(B)]
    in_maps = []
    for c in range(NCORES):
        b, g = c // GROUPS, c % GROUPS
        sl = slice(g * GH, (g + 1) * GH)
        bqk = np.stack([(ESC * bq[sl]).reshape(OT, 128).T,
                        (ESC * bk[sl]).reshape(OT, 128).T],
                       1).reshape(128, 2 * OT)
        in_maps.append({
            "xq": xq[b], "xk": xk[b], "xv": xv[b],
            "wq": np.ascontiguousarray(ESC * Wq[sl, :].T).astype(E4),
            "wk": np.ascontiguousarray(ESC * Wk[sl, :].T).astype(E4),
            "wv": np.ascontiguousarray(ESC * Wv[sl, :].T).astype(E4),
            "bqk": np.ascontiguousarray(bqk, dtype=np.float32),
            "bv": np.ascontiguousarray(ESC * bv[None, sl]).astype(E4),
            "onesd": np.ones((128, 128), E4),
        })
    return in_maps


def run(inputs, mode=MODE, trace=False):
    nc = _get_nc(mode)
    in_maps = _prep_inputs(inputs)
    res = bass_utils.run_bass_kernel_spmd(
        nc, in_maps, core_ids=list(range(NCORES)), trace=trace)

    masks = np.asarray(inputs["masks"], np.float32)
    query = np.asarray(inputs["query"], np.float32)
    out = np.empty((B, S, H), np.float32)
    for c in range(NCORES):
        b, g = c // GROUPS, c % GROUPS
        hid = res.results[c]["hid"].reshape(HL, DH + 1, S)
        hT = hid[:, :DH, :]                      # (HL, DH, S)  (64x scaled)
        se = hid[:, DH, :]                       # (HL, S)
        blk = (hT / (ESC * se[:, None, :])).transpose(2, 0, 1).reshape(S, GH)
        out[b, :, g * GH:(g + 1) * GH] = blk
    out = out * masks[:, :, None] + query
    return out, res


def kernel(**inputs) -> np.ndarray:
    out, _ = run(inputs)
    return out


# revision 15
# speedup vs baseline: 1.3601x; 1.0214x over previous
"""Multi-head attention (ReLU-gated projections) on 8 Trainium2 NeuronCores.

Problem (hardcoded): B=4, S=1024, H=1024, NH=16, DH=64.
  qp = relu(q @ Wq.T + bq); kp, vp likewise
  alpha = softmax(qh @ kh.T / sqrt(DH)) * mask[q]
  out = (alpha @ vh).reshape(B,S,H) + query

Sharding: 8 cores = 4 batches x 2 head-groups (8 heads / 512 hidden cols each).

fp8 design: all matmuls in fp8 e4m3 (TRN2 flavor: with-inf, max finite 240).
Weights pre-scaled by 32 on the host so their N(0, 1/32) values use e4m3's
normal range; the 32x factors ride through the linear pipeline (qp,kp,vp
all carry 32x) and are compensated in the exp scale (1/(8*32^2)) and a
final /32 on the host. exp also subtracts 3.0 (cancels in softmax) to
keep pt under the 240 cap. Projections and AV use MatmulPerfMode.DoubleRow
(K=256 per instruction, 2x PE throughput); alpha matmuls are
output-rate-bound so they stay plain fp8 with the kz zero-padded-K trick.
The AV stationary keeps a ones column (M=65) so row 64 accumulates sumexp
for free; the per-head V slot is padded to 68 bytes so DoubleRow weight
APs stay 4-byte aligned (ISA restriction s3_lw_dual_fp8).

Host pre-arranges x/w into the exact SBUF layouts so every input DMA is
128 partitions x 4KB contiguous. Consts load first so the PE-clock warmup
(and the ACT exp-table preload) start immediately.

Per-core device kernel (transposed "hidden-on-partitions" layout):
  stage 1: qpT[o,s], kpT[o,s] (transposed) and vp[s,o] (normal) projections
           with fused bias+relu, evacuated to fp8.
  stage 2: per head: alphaT[k,q] psum tiles; pt = exp(alpha*sc - 3) in fp8
           written into paired [128, 2048] tiles; AV via DoubleRow with the
           ones column -> unnormalized hidT (64,S) + sumexp (S) per head;
           host divides, applies mask, adds residual.
"""
import sys

sys.path.insert(0, "/opt/trn_rl_repo")

import os
import numpy as np
import ml_dtypes

import concourse.bass as bass
import concourse.tile as tile
from concourse import bacc, mybir
from concourse import bass_utils

B, S, H = 4, 1024, 1024
NH, DH = 16, 64
NCORES = 8
GROUPS = 2          # head-groups (tensor-parallel dim)
HL = NH // GROUPS   # heads per core = 8
GH = H // GROUPS    # hidden cols per core = 512
KT = H // 128       # contraction k-tiles = 8
OT = GH // 128      # output o-tiles per core = 4
SCALE = 1.0 / float(np.sqrt(DH))
ESC = 32.0          # fp8 weight pre-scale (TRN2 fp8e4 = e4m3-with-inf,
                    # max finite 240: keep relu'd projections under ~170)
VW8 = HL * 68       # padded per-head v slot (64 v + 1 ones + 3 pad) = 544

MODE = os.environ.get("BASS_MM_DT", "fp8")

F32 = mybir.dt.float32
BF16 = mybir.dt.bfloat16
FP8 = mybir.dt.float8e4
DR = mybir.MatmulPerfMode.DoubleRow
E4 = ml_dtypes.float8_e4m3   # e4m3 WITH inf (max 240) — matches TRN2 hw


def build_fp8():
    nc = bacc.Bacc("TRN2", target_bir_lowering=False, debug=False,
                   num_devices=NCORES)

    # x/w arrive pre-arranged in SBUF layout: [128, KT*512] per s-chunk
    x_d = {(w, sc): nc.dram_tensor(f"x{w}{sc}", [128, KT * 512], FP8,
                                   kind="ExternalInput").ap()
           for w in "qkv" for sc in range(2)}
    w_d = {w: nc.dram_tensor(f"w{w}", [128, KT * GH], FP8,
                             kind="ExternalInput").ap()
           for w in "qkv"}
    bqk_d = nc.dram_tensor("bqk", [128, 2 * OT], F32, kind="ExternalInput").ap()
    bv_d = nc.dram_tensor("bv", [1, GH], FP8, kind="ExternalInput").ap()
    ones_d = nc.dram_tensor("onesd", [128, 128], FP8,
                            kind="ExternalInput").ap()
    hid_d = nc.dram_tensor("hid", [HL * (DH + 1), S], F32,
                           kind="ExternalOutput").ap()

    EXP_SCALE = SCALE / (ESC * ESC)
    EXP_BIAS = -3.0   # pt = exp(alpha/8 - 3): keeps exp under e4m3 max 240;
                      # cancels in hid/sumexp

    with tile.TileContext(nc) as tc:
        with tc.tile_pool(name="sb", bufs=1) as sb, \
             tc.tile_pool(name="ps", bufs=1, space="PSUM") as ps:

            # ---- persistent tiles ----
            wq_t = sb.tile([128, KT * GH], FP8, tag="wq", name="wq")
            wk_t = sb.tile([128, KT * GH], FP8, tag="wk", name="wk")
            wv_t = sb.tile([128, KT * GH], FP8, tag="wv", name="wv")
            qp_t = [sb.tile([128, S], FP8, tag=f"qp{t}", name=f"qp{t}")
                    for t in range(OT)]
            kz_t = [[sb.tile([128, S], FP8, tag=f"kz{t}{h}",
                             name=f"kz{t}{h}") for h in range(2)]
                    for t in range(OT)]
            kz_zeroed = set()
            vp_t = sb.tile([128, KT * VW8], FP8, tag="vp", name="vp")
            bqk_t = sb.tile([128, 2 * OT], F32, tag="bqk", name="bqk")
            bv_t = sb.tile([1, GH], FP8, tag="bv", name="bv")
            ones_t = sb.tile([1, 128], FP8, tag="ones", name="ones")
            ones64_t = sb.tile([128, KT * HL], FP8, tag="ones64",
                               name="ones64")
            expb_t = sb.tile([128, 1], F32, tag="expb", name="expb")
            nc.vector.memset(expb_t[:], EXP_BIAS)

            # ---- loads: consts first (they gate warmup + exp-table preload),
            #      then x/w round-robin over 4 DGE rings in need order ----
            nc.sync.dma_start(bqk_t[:], bqk_d)
            nc.sync.dma_start(bv_t[:], bv_d)
            nc.sync.dma_start(ones_t[:], ones_d[0:1, :])
            nc.sync.dma_start(ones64_t[:], ones_d[:, 0:KT * HL])

            x_t = {}

            def x_ld(which, sc, eng):
                t = sb.tile([128, KT * 512], FP8, tag=f"x{which}{sc}",
                            name=f"x{which}_{sc}")
                eng.dma_start(t[:], x_d[(which, sc)])
                x_t[(which, sc)] = t

            def x3(which, sc):
                return x_t[(which, sc)][:].rearrange("p (k s) -> p k s", s=512)

            x_ld("q", 0, nc.scalar)
            x_ld("k", 0, nc.gpsimd)
            nc.sync.dma_start(wq_t[:], w_d["q"])
            nc.scalar.dma_start(wk_t[:], w_d["k"])
            x_ld("q", 1, nc.gpsimd)
            x_ld("k", 1, nc.sync)
            nc.gpsimd.dma_start(wv_t[:], w_d["v"])
            x_ld("v", 0, nc.sync)
            x_ld("v", 1, nc.scalar)

            # HAM warmup: ramp the PE clock while the x/w DMAs land
            warm = ps.tile([65, 512], F32, tag="av", bufs=2, name="warm")
            for i in range(16):
                nc.tensor.matmul(warm[:], ones_t[:, 0:65], bv_t[:],
                                 start=True, stop=True)
            # preload the ACT exp table while ACT is idle
            dummy_exp = sb.tile([1, 8], F32, tag="dummy_exp", name="dummy_exp")
            nc.scalar.activation(dummy_exp[:], ones_t[0:1, 0:8],
                                 mybir.ActivationFunctionType.Exp, scale=1.0)

            # ones column of the AV stationary
            v4 = vp_t[:].rearrange("p (k n c) -> p k n c", n=HL, c=68)
            nc.vector.tensor_copy(
                v4[:, :, :, DH:DH + 1],
                ones64_t[:].rearrange("p (k n one) -> p k n one", n=HL, one=1))

            def proj_qk(sc, ot, which):
                """one o-tile, one s-chunk of the transposed q/k projection"""
                w_t = wq_t if which == "q" else wk_t
                pp = ps.tile([128, 1024], F32, tag="alpha", bufs=3,
                             name=f"pp{which}_{sc}_{ot}")
                w3 = w_t[:].rearrange("p (k o) -> p k o", o=GH)
                xv_ = x3(which, sc)
                for kp in range(KT // 2):
                    nc.tensor.matmul(
                        pp[:, 0:512],
                        w3[:, 2 * kp:2 * kp + 2, ot * 128:(ot + 1) * 128],
                        xv_[:, 2 * kp:2 * kp + 2, :],
                        start=(kp == 0), stop=(kp == KT // 2 - 1),
                        perf_mode=DR)
                wi = 0 if which == "q" else 1
                bias = bqk_t[:, wi * OT + ot:wi * OT + ot + 1]
                ssl = slice(sc * 512, (sc + 1) * 512)
                if which == "q":
                    nc.vector.tensor_scalar(
                        qp_t[ot][:, ssl], pp[:, 0:512], bias, 0.0,
                        mybir.AluOpType.add, mybir.AluOpType.max)
                else:
                    for h in range(2):
                        pr = slice(h * 64, h * 64 + 64)
                        nc.vector.tensor_scalar(
                            kz_t[ot][h][pr, ssl], pp[pr, 0:512], bias[pr, :],
                            0.0, mybir.AluOpType.add, mybir.AluOpType.max)

            def proj_v(sc, j):
                """one s-tile (128 rows of vp) within chunk sc"""
                st = sc * 4 + j
                pp = ps.tile([128, 1024], F32, tag="alpha", bufs=3,
                             name=f"ppv_{st}")
                nc.tensor.matmul(pp[:, 0:512], ones_t[:], bv_t[:],
                                 start=True, stop=False)
                wv3 = wv_t[:].rearrange("p (k o) -> p k o", o=GH)
                xv_ = x3("v", sc)
                for kp in range(KT // 2):
                    nc.tensor.matmul(
                        pp[:, 0:512],
                        xv_[:, 2 * kp:2 * kp + 2, j * 128:(j + 1) * 128],
                        wv3[:, 2 * kp:2 * kp + 2, :],
                        start=False, stop=(kp == KT // 2 - 1),
                        perf_mode=DR)
                v3 = vp_t[:, st * VW8:(st + 1) * VW8].rearrange(
                    "p (n c) -> p n c", c=68)
                p3 = pp[:, 0:512].rearrange("p (n c) -> p n c", c=DH)
                nc.vector.tensor_scalar(
                    v3[:, :, 0:DH], p3, 0.0, None, mybir.AluOpType.max)

            pt_all = {}

            def alphas(n0):
                """alpha + exp for head pair (n0, n0+1), head-major so each
                head's pt tiles complete early and its AV can start while the
                other head's exps still stream.  pt tiles are paired
                [128, 2048] (two k-tiles) so AV consumes them via DoubleRow."""
                t = n0 // 2
                if t not in kz_zeroed:
                    kz_zeroed.add(t)
                    nc.gpsimd.memset(kz_t[t][0][64:128, :], 0.0)
                    nc.gpsimd.memset(kz_t[t][1][0:64, :], 0.0)
                for h in range(2):
                    pts = []
                    cur = None
                    for k in range(KT):
                        apt = ps.tile([128, 1024], F32, tag="alpha", bufs=3,
                                      name=f"alp_{n0 + h}_{k}")
                        for qc in range(2):
                            nc.tensor.matmul(
                                apt[:, qc * 512:(qc + 1) * 512],
                                kz_t[t][h][:, k * 128:(k + 1) * 128],
                                qp_t[t][:, qc * 512:(qc + 1) * 512],
                                start=True, stop=True)
                        half = k % 2
                        if half == 0:
                            cur = sb.tile([128, 2048], FP8, tag="pt",
                                          bufs=16, name=f"pt_{n0 + h}_{k}")
                            pts.append(cur)
                        nc.scalar.activation(
                            cur[:, half * 1024:(half + 1) * 1024], apt[:],
                            mybir.ActivationFunctionType.Exp, scale=EXP_SCALE,
                            bias=expb_t[:])
                    pt_all[n0 + h] = pts

            def avs(n):
                pts = pt_all.pop(n)
                hid_t = sb.tile([DH + 1, S], F32, tag="hid",
                                bufs=3, name=f"hid_{n}")
                for qc in range(2):
                    av = ps.tile([DH + 1, 512], F32, tag="av", bufs=2,
                                 name=f"av_{n}_{qc}")
                    for kp in range(KT // 2):
                        nc.tensor.matmul(
                            av[:],
                            v4[:, 2 * kp:2 * kp + 2, n, 0:DH + 1],
                            pts[kp][:].rearrange(
                                "p (k s) -> p k s",
                                s=1024)[:, :, qc * 512:(qc + 1) * 512],
                            start=(kp == 0), stop=(kp == KT // 2 - 1),
                            perf_mode=DR)
                    nc.vector.tensor_copy(
                        hid_t[:, qc * 512:(qc + 1) * 512], av[:])
                    nc.sync.dma_start(
                        hid_d[n * (DH + 1):(n + 1) * (DH + 1),
                              qc * 512:(qc + 1) * 512],
                        hid_t[:, qc * 512:(qc + 1) * 512])

            # ---- emission schedule: keep the ACT exp stream (the
            #      co-bottleneck) hot from ~alphas(0) to the end ----
            proj_qk(0, 0, "q")
            proj_qk(0, 0, "k")
            proj_qk(1, 0, "q")
            proj_qk(1, 0, "k")
            alphas(0)
            proj_qk(0, 1, "q")
            proj_qk(0, 1, "k")
            proj_qk(1, 1, "q")
            proj_qk(1, 1, "k")
            alphas(2)
            for j in range(4):
                proj_v(0, j)
            for j in range(4):
                proj_v(1, j)
            avs(0)
            avs(1)
            proj_qk(0, 2, "q")
            proj_qk(0, 2, "k")
            proj_qk(1, 2, "q")
            proj_qk(1, 2, "k")
            alphas(4)
            avs(2)
            avs(3)
            proj_qk(0, 3, "q")
            proj_qk(0, 3, "k")
            proj_qk(1, 3, "q")
            proj_qk(1, 3, "k")
            alphas(6)
            avs(4)
            avs(5)
            avs(6)
            avs(7)

    nc.compile()
    return nc


_NC_CACHE = {}


def _get_nc(mode):
    if mode not in _NC_CACHE:
        if mode != "fp8":
            raise ValueError(f"unsupported mode {mode}")
        _NC_CACHE[mode] = build_fp8()
    return _NC_CACHE[mode]


def _sbuf_layout_x(xT):
    """[H, S] transposed input -> per-chunk [128, KT*512] SBUF image"""
    x4 = xT.reshape(KT, 128, 2, 512)          # [k, p, sc, s]
    return [np.ascontiguousarray(
        x4[:, :, sc, :].transpose(1, 0, 2).reshape(128, KT * 512)).astype(E4)
        for sc in range(2)]


def _sbuf_layout_w(wT):
    """[H, GH] transposed weight -> [128, KT*GH] SBUF image"""
    w3 = wT.reshape(KT, 128, GH)
    return np.ascontiguousarray(
        w3.transpose(1, 0, 2).reshape(128, KT * GH)).astype(E4)


def _prep_inputs(inputs):
    q = np.asarray(inputs["query"], np.float32)
    k = np.asarray(inputs["key"], np.float32)
    v = np.asarray(inputs["value"], np.float32)
    Wq = np.asarray(inputs["Wq"], np.float32)
    Wk = np.asarray(inputs["Wk"], np.float32)
    Wv = np.asarray(inputs["Wv"], np.float32)
    bq = np.asarray(inputs["bq"], np.float32)
    bk = np.asarray(inputs["bk"], np.float32)
    bv = np.asarray(inputs["bv"], np.float32)

    xq = [_sbuf_layout_x(q[b].T) for b in range(B)]
    xk = [_sbuf_layout_x(k[b].T) for b in range(B)]
    xv = [_sbuf_layout_x(v[b].T) for b in range(B)]
    in_maps = []
    for c in range(NCORES):
        b, g = c // GROUPS, c % GROUPS
        sl = slice(g * GH, (g + 1) * GH)
        bqk = np.stack([(ESC * bq[sl]).reshape(OT, 128).T,
                        (ESC * bk[sl]).reshape(OT, 128).T],
                       1).reshape(128, 2 * OT)
        in_maps.append({
            "xq0": xq[b][0], "xq1": xq[b][1],
            "xk0": xk[b][0], "xk1": xk[b][1],
            "xv0": xv[b][0], "xv1": xv[b][1],
            "wq": _sbuf_layout_w(ESC * Wq[sl, :].T),
            "wk": _sbuf_layout_w(ESC * Wk[sl, :].T),
            "wv": _sbuf_layout_w(ESC * Wv[sl, :].T),
            "bqk": np.ascontiguousarray(bqk, dtype=np.float32),
            "bv": np.ascontiguousarray(ESC * bv[None, sl]).astype(E4),
            "onesd": np.ones((128, 128), E4),
        })
    return in_maps


def run(inputs, mode=MODE, trace=False):
    nc = _get_nc(mode)
    in_maps = _prep_inputs(inputs)
    res = bass_utils.run_bass_kernel_spmd(
        nc, in_maps, core_ids=list(range(NCORES)), trace=trace)

    masks = np.asarray(inputs["masks"], np.float32)
    query = np.asarray(inputs["query"], np.float32)
    out = np.empty((B, S, H), np.float32)
    for c in range(NCORES):
        b, g = c // GROUPS, c % GROUPS
        hid = res.results[c]["hid"].reshape(HL, DH + 1, S)
        hT = hid[:, :DH, :]                      # (HL, DH, S)  (32x scaled)
        se = hid[:, DH, :]                       # (HL, S)
        blk = (hT / (ESC * se[:, None, :])).transpose(2, 0, 1).reshape(S, GH)
        out[b, :, g * GH:(g + 1) * GH] = blk
    out = out * masks[:, :, None] + query
    return out, res


def kernel(**inputs) -> np.ndarray:
    out, _ = run(inputs)
    return out


# revision 19
# speedup vs baseline: 1.3602x; 1.0000x over previous
"""Multi-head attention (ReLU-gated projections) on 8 Trainium2 NeuronCores.

Problem (hardcoded): B=4, S=1024, H=1024, NH=16, DH=64.
  qp = relu(q @ Wq.T + bq); kp, vp likewise
  alpha = softmax(qh @ kh.T / sqrt(DH)) * mask[q]
  out = (alpha @ vh).reshape(B,S,H) + query

Sharding: 8 cores = 4 batches x 2 head-groups (8 heads / 512 hidden cols each).

fp8 design: all matmuls in fp8 e4m3 (TRN2 flavor: with-inf, max finite 240).
Weights pre-scaled by 32 on the host so their N(0, 1/32) values use e4m3's
normal range; the 32x factors ride through the linear pipeline (qp,kp,vp
all carry 32x) and are compensated in the exp scale (1/(8*32^2)) and a
final /32 on the host. exp also subtracts 3.0 (cancels in softmax) to
keep pt under the 240 cap. Projections and AV use MatmulPerfMode.DoubleRow
(K=256 per instruction, 2x PE throughput); alpha matmuls are
output-rate-bound so they stay plain fp8 with the kz zero-padded-K trick.
The AV stationary keeps a ones column (M=65) so row 64 accumulates sumexp
for free; the per-head V slot is padded to 68 bytes so DoubleRow weight
APs stay 4-byte aligned (ISA restriction s3_lw_dual_fp8).

Host pre-arranges x/w into the exact SBUF layouts so every input DMA is
128 partitions x 4KB contiguous. Consts load first so the PE-clock warmup
(and the ACT exp-table preload) start immediately.

Per-core device kernel (transposed "hidden-on-partitions" layout):
  stage 1: qpT[o,s], kpT[o,s] (transposed) and vp[s,o] (normal) projections
           with fused bias+relu, evacuated to fp8.
  stage 2: per head: alphaT[k,q] psum tiles; pt = exp(alpha*sc - 3) in fp8
           written into paired [128, 2048] tiles; AV via DoubleRow with the
           ones column -> unnormalized hidT (64,S) + sumexp (S) per head;
           host divides, applies mask, adds residual.
"""
import sys

sys.path.insert(0, "/opt/trn_rl_repo")

import os
import numpy as np
import ml_dtypes

import concourse.bass as bass
import concourse.tile as tile
from concourse import bacc, mybir
from concourse import bass_utils

B, S, H = 4, 1024, 1024
NH, DH = 16, 64
NCORES = 8
GROUPS = 2          # head-groups (tensor-parallel dim)
HL = NH // GROUPS   # heads per core = 8
GH = H // GROUPS    # hidden cols per core = 512
KT = H // 128       # contraction k-tiles = 8
OT = GH // 128      # output o-tiles per core = 4
SCALE = 1.0 / float(np.sqrt(DH))
ESC = 32.0          # fp8 weight pre-scale (TRN2 fp8e4 = e4m3-with-inf,
                    # max finite 240: keep relu'd projections under ~170)
VW8 = HL * 68       # padded per-head v slot (64 v + 1 ones + 3 pad) = 544

MODE = os.environ.get("BASS_MM_DT", "fp8")

F32 = mybir.dt.float32
BF16 = mybir.dt.bfloat16
FP8 = mybir.dt.float8e4
DR = mybir.MatmulPerfMode.DoubleRow
E4 = ml_dtypes.float8_e4m3   # e4m3 WITH inf (max 240) — matches TRN2 hw


def build_fp8():
    nc = bacc.Bacc("TRN2", target_bir_lowering=False, debug=False,
                   num_devices=NCORES)

    # x/w arrive pre-arranged in SBUF layout: [128, KT*512] per s-chunk
    x_d = {(w, sc): nc.dram_tensor(f"x{w}{sc}", [128, KT * 512], FP8,
                                   kind="ExternalInput").ap()
           for w in "qkv" for sc in range(2)}
    w_d = {w: nc.dram_tensor(f"w{w}", [128, KT * GH], FP8,
                             kind="ExternalInput").ap()
           for w in "qkv"}
    bqk_d = nc.dram_tensor("bqk", [128, 2 * OT], F32, kind="ExternalInput").ap()
    bv_d = nc.dram_tensor("bv", [1, GH], FP8, kind="ExternalInput").ap()
    ones_d = nc.dram_tensor("onesd", [128, 128], FP8,
                            kind="ExternalInput").ap()
    hid_d = nc.dram_tensor("hid", [HL * (DH + 1), S], F32,
                           kind="ExternalOutput").ap()

    EXP_SCALE = SCALE / (ESC * ESC)
    EXP_BIAS = -3.0   # pt = exp(alpha/8 - 3): keeps exp under e4m3 max 240;
                      # cancels in hid/sumexp

    with tile.TileContext(nc) as tc:
        with tc.tile_pool(name="sb", bufs=1) as sb, \
             tc.tile_pool(name="ps", bufs=1, space="PSUM") as ps:

            # ---- persistent tiles ----
            wq_t = sb.tile([128, KT * GH], FP8, tag="wq", name="wq")
            wk_t = sb.tile([128, KT * GH], FP8, tag="wk", name="wk")
            wv_t = sb.tile([128, KT * GH], FP8, tag="wv", name="wv")
            qp_t = [sb.tile([128, S], FP8, tag=f"qp{t}", name=f"qp{t}")
                    for t in range(OT)]
            kz_t = [[sb.tile([128, S], FP8, tag=f"kz{t}{h}",
                             name=f"kz{t}{h}") for h in range(2)]
                    for t in range(OT)]
            kz_zeroed = set()
            vp_t = sb.tile([128, KT * VW8], FP8, tag="vp", name="vp")
            bqk_t = sb.tile([128, 2 * OT], F32, tag="bqk", name="bqk")
            bv_t = sb.tile([1, GH], FP8, tag="bv", name="bv")
            ones_t = sb.tile([1, 128], FP8, tag="ones", name="ones")
            ones64_t = sb.tile([128, KT * HL], FP8, tag="ones64",
                               name="ones64")
            expb_t = sb.tile([128, 1], F32, tag="expb", name="expb")
            nc.vector.memset(expb_t[:], EXP_BIAS)

            # ---- warmup from memset tiles: no DMA dependency, so the PE
            #      clock ramp and the ACT exp-table preload start at ~1us ----
            wstat = sb.tile([1, 128], FP8, tag="wstat", name="wstat")
            wmov = sb.tile([1, 512], FP8, tag="wmov", name="wmov")
            nc.vector.memset(wstat[:], 1.0)
            nc.vector.memset(wmov[:], 1.0)
            warm = ps.tile([65, 512], F32, tag="av", bufs=2, name="warm")
            for i in range(20):
                nc.tensor.matmul(warm[:], wstat[:, 0:65], wmov[:],
                                 start=True, stop=True)
            dummy_exp = sb.tile([1, 8], F32, tag="dummy_exp", name="dummy_exp")
            nc.scalar.activation(dummy_exp[:], wmov[0:1, 0:8],
                                 mybir.ActivationFunctionType.Exp, scale=1.0)

            # ---- loads: consts first, then x/w in fine k-pair slices
            #      (128KB each) round-robin over the 3 DGE rings in need
            #      order, so projections start as soon as slices land ----
            nc.sync.dma_start(bqk_t[:], bqk_d)
            nc.scalar.dma_start(bv_t[:], bv_d)
            nc.gpsimd.dma_start(ones_t[:], ones_d[0:1, :])
            nc.sync.dma_start(ones64_t[:], ones_d[:, 0:KT * HL])

            x_t = {}
            rings = [nc.sync, nc.scalar, nc.gpsimd]
            ring_i = [0]

            def ld_sliced(dst_t, src_d, n_slices=4):
                w = dst_t.shape[1] // n_slices
                for s in range(n_slices):
                    eng = rings[ring_i[0] % 3]
                    ring_i[0] += 1
                    eng.dma_start(dst_t[:, s * w:(s + 1) * w],
                                  src_d[:, s * w:(s + 1) * w])

            def x_ld(which, sc):
                t = sb.tile([128, KT * 512], FP8, tag=f"x{which}{sc}",
                            name=f"x{which}_{sc}")
                x_t[(which, sc)] = t
                ld_sliced(t, x_d[(which, sc)])

            def x3(which, sc):
                return x_t[(which, sc)][:].rearrange("p (k s) -> p k s", s=512)

            x_ld("q", 0)
            ld_sliced(wq_t, w_d["q"])
            x_ld("k", 0)
            ld_sliced(wk_t, w_d["k"])
            x_ld("q", 1)
            x_ld("k", 1)
            x_ld("v", 0)
            ld_sliced(wv_t, w_d["v"])
            x_ld("v", 1)

            # ones column of the AV stationary
            v4 = vp_t[:].rearrange("p (k n c) -> p k n c", n=HL, c=68)
            nc.vector.tensor_copy(
                v4[:, :, :, DH:DH + 1],
                ones64_t[:].rearrange("p (k n one) -> p k n one", n=HL, one=1))

            def proj_qk(sc, ot, which):
                """one o-tile, one s-chunk of the transposed q/k projection"""
                w_t = wq_t if which == "q" else wk_t
                pp = ps.tile([128, 1024], F32, tag="alpha", bufs=3,
                             name=f"pp{which}_{sc}_{ot}")
                w3 = w_t[:].rearrange("p (k o) -> p k o", o=GH)
                xv_ = x3(which, sc)
                for kp in range(KT // 2):
                    nc.tensor.matmul(
                        pp[:, 0:512],
                        w3[:, 2 * kp:2 * kp + 2, ot * 128:(ot + 1) * 128],
                        xv_[:, 2 * kp:2 * kp + 2, :],
                        start=(kp == 0), stop=(kp == KT // 2 - 1),
                        perf_mode=DR)
                wi = 0 if which == "q" else 1
                bias = bqk_t[:, wi * OT + ot:wi * OT + ot + 1]
                ssl = slice(sc * 512, (sc + 1) * 512)
                if which == "q":
                    nc.vector.tensor_scalar(
                        qp_t[ot][:, ssl], pp[:, 0:512], bias, 0.0,
                        mybir.AluOpType.add, mybir.AluOpType.max)
                else:
                    for h in range(2):
                        pr = slice(h * 64, h * 64 + 64)
                        nc.vector.tensor_scalar(
                            kz_t[ot][h][pr, ssl], pp[pr, 0:512], bias[pr, :],
                            0.0, mybir.AluOpType.add, mybir.AluOpType.max)

            def proj_v(sc, j):
                """one s-tile (128 rows of vp) within chunk sc"""
                st = sc * 4 + j
                pp = ps.tile([128, 1024], F32, tag="alpha", bufs=3,
                             name=f"ppv_{st}")
                nc.tensor.matmul(pp[:, 0:512], ones_t[:], bv_t[:],
                                 start=True, stop=False)
                wv3 = wv_t[:].rearrange("p (k o) -> p k o", o=GH)
                xv_ = x3("v", sc)
                for kp in range(KT // 2):
                    nc.tensor.matmul(
                        pp[:, 0:512],
                        xv_[:, 2 * kp:2 * kp + 2, j * 128:(j + 1) * 128],
                        wv3[:, 2 * kp:2 * kp + 2, :],
                        start=False, stop=(kp == KT // 2 - 1),
                        perf_mode=DR)
                v3 = vp_t[:, st * VW8:(st + 1) * VW8].rearrange(
                    "p (n c) -> p n c", c=68)
                p3 = pp[:, 0:512].rearrange("p (n c) -> p n c", c=DH)
                nc.vector.tensor_scalar(
                    v3[:, :, 0:DH], p3, 0.0, None, mybir.AluOpType.max)

            pt_all = {}

            def alphas(n0):
                """alpha + exp for head pair (n0, n0+1), head-major so each
                head's pt tiles complete early and its AV can start while the
                other head's exps still stream.  pt tiles are paired
                [128, 2048] (two k-tiles) so AV consumes them via DoubleRow."""
                t = n0 // 2
                if t not in kz_zeroed:
                    kz_zeroed.add(t)
                    nc.gpsimd.memset(kz_t[t][0][64:128, :], 0.0)
                    nc.gpsimd.memset(kz_t[t][1][0:64, :], 0.0)
                for h in range(2):
                    pts = []
                    cur = None
                    for k in range(KT):
                        apt = ps.tile([128, 1024], F32, tag="alpha", bufs=3,
                                      name=f"alp_{n0 + h}_{k}")
                        for qc in range(2):
                            nc.tensor.matmul(
                                apt[:, qc * 512:(qc + 1) * 512],
                                kz_t[t][h][:, k * 128:(k + 1) * 128],
                                qp_t[t][:, qc * 512:(qc + 1) * 512],
                                start=True, stop=True)
                        half = k % 2
                        if half == 0:
                            cur = sb.tile([128, 2048], FP8, tag="pt",
                                          bufs=16, name=f"pt_{n0 + h}_{k}")
                            pts.append(cur)
                        nc.scalar.activation(
                            cur[:, half * 1024:(half + 1) * 1024], apt[:],
                            mybir.ActivationFunctionType.Exp, scale=EXP_SCALE,
                            bias=expb_t[:])
                    pt_all[n0 + h] = pts

            def avs(n):
                pts = pt_all.pop(n)
                hid_t = sb.tile([DH + 1, S], F32, tag="hid",
                                bufs=3, name=f"hid_{n}")
                for qc in range(2):
                    av = ps.tile([DH + 1, 512], F32, tag="av", bufs=2,
                                 name=f"av_{n}_{qc}")
                    for kp in range(KT // 2):
                        nc.tensor.matmul(
                            av[:],
                            v4[:, 2 * kp:2 * kp + 2, n, 0:DH + 1],
                            pts[kp][:].rearrange(
                                "p (k s) -> p k s",
                                s=1024)[:, :, qc * 512:(qc + 1) * 512],
                            start=(kp == 0), stop=(kp == KT // 2 - 1),
                            perf_mode=DR)
                    nc.vector.tensor_copy(
                        hid_t[:, qc * 512:(qc + 1) * 512], av[:])
                    eng = rings[ring_i[0] % 3]
                    ring_i[0] += 1
                    eng.dma_start(
                        hid_d[n * (DH + 1):(n + 1) * (DH + 1),
                              qc * 512:(qc + 1) * 512],
                        hid_t[:, qc * 512:(qc + 1) * 512])

            # ---- emission schedule: keep the ACT exp stream (the
            #      co-bottleneck) hot from ~alphas(0) to the end ----
            proj_qk(0, 0, "q")
            proj_qk(0, 0, "k")
            proj_qk(1, 0, "q")
            proj_qk(1, 0, "k")
            alphas(0)
            proj_qk(0, 1, "q")
            proj_qk(0, 1, "k")
            proj_qk(1, 1, "q")
            proj_qk(1, 1, "k")
            proj_v(0, 0)
            proj_v(0, 1)
            alphas(2)
            proj_v(0, 2)
            proj_v(0, 3)
            for j in range(4):
                proj_v(1, j)
            avs(0)
            avs(1)
            proj_qk(0, 2, "q")
            proj_qk(0, 2, "k")
            proj_qk(1, 2, "q")
            proj_qk(1, 2, "k")
            alphas(4)
            avs(2)
            avs(3)
            proj_qk(0, 3, "q")
            proj_qk(0, 3, "k")
            proj_qk(1, 3, "q")
            proj_qk(1, 3, "k")
            alphas(6)
            avs(4)
            avs(5)
            avs(6)
            avs(7)

    nc.compile()
    return nc


_NC_CACHE = {}


def _get_nc(mode):
    if mode not in _NC_CACHE:
        if mode != "fp8":
            raise ValueError(f"unsupported mode {mode}")
        _NC_CACHE[mode] = build_fp8()
    return _NC_CACHE[mode]


def _sbuf_layout_x(xT):
    """[H, S] transposed input -> per-chunk [128, KT*512] SBUF image"""
    x4 = xT.reshape(KT, 128, 2, 512)          # [k, p, sc, s]
    return [np.ascontiguousarray(
        x4[:, :, sc, :].transpose(1, 0, 2).reshape(128, KT * 512)).astype(E4)
        for sc in range(2)]


def _sbuf_layout_w(wT):
    """[H, GH] transposed weight -> [128, KT*GH] SBUF image"""
    w3 = wT.reshape(KT, 128, GH)
    return np.ascontiguousarray(
        w3.transpose(1, 0, 2).reshape(128, KT * GH)).astype(E4)


def _prep_inputs(inputs):
    q = np.asarray(inputs["query"], np.float32)
    k = np.asarray(inputs["key"], np.float32)
    v = np.asarray(inputs["value"], np.float32)
    Wq = np.asarray(inputs["Wq"], np.float32)
    Wk = np.asarray(inputs["Wk"], np.float32)
    Wv = np.asarray(inputs["Wv"], np.float32)
    bq = np.asarray(inputs["bq"], np.float32)
    bk = np.asarray(inputs["bk"], np.float32)
    bv = np.asarray(inputs["bv"], np.float32)

    xq = [_sbuf_layout_x(q[b].T) for b in range(B)]
    xk = [_sbuf_layout_x(k[b].T) for b in range(B)]
    xv = [_sbuf_layout_x(v[b].T) for b in range(B)]
    in_maps = []
    for c in range(NCORES):
        b, g = c // GROUPS, c % GROUPS
        sl = slice(g * GH, (g + 1) * GH)
        bqk = np.stack([(ESC * bq[sl]).reshape(OT, 128).T,
                        (ESC * bk[sl]).reshape(OT, 128).T],
                       1).reshape(128, 2 * OT)
        in_maps.append({
            "xq0": xq[b][0], "xq1": xq[b][1],
            "xk0": xk[b][0], "xk1": xk[b][1],
            "xv0": xv[b][0], "xv1": xv[b][1],
            "wq": _sbuf_layout_w(ESC * Wq[sl, :].T),
            "wk": _sbuf_layout_w(ESC * Wk[sl, :].T),
            "wv": _sbuf_layout_w(ESC * Wv[sl, :].T),
            "bqk": np.ascontiguousarray(bqk, dtype=np.float32),
            "bv": np.ascontiguousarray(ESC * bv[None, sl]).astype(E4),
            "onesd": np.ones((128, 128), E4),
        })
    return in_maps


def run(inputs, mode=MODE, trace=False):
    nc = _get_nc(mode)
    in_maps = _prep_inputs(inputs)
    res = bass_utils.run_bass_kernel_spmd(
        nc, in_maps, core_ids=list(range(NCORES)), trace=trace)

    masks = np.asarray(inputs["masks"], np.float32)
    query = np.asarray(inputs["query"], np.float32)
    out = np.empty((B, S, H), np.float32)
    for c in range(NCORES):
        b, g = c // GROUPS, c % GROUPS
        hid = res.results[c]["hid"].reshape(HL, DH + 1, S)
        hT = hid[:, :DH, :]                      # (HL, DH, S)  (32x scaled)
        se = hid[:, DH, :]                       # (HL, S)
        blk = (hT / (ESC * se[:, None, :])).transpose(2, 0, 1).reshape(S, GH)
        out[b, :, g * GH:(g + 1) * GH] = blk
    out = out * masks[:, :, None] + query
    return out, res


def kernel(**inputs) -> np.ndarray:
    out, _ = run(inputs)
    return out


# revision 22
# speedup vs baseline: 1.3899x; 1.0218x over previous
"""Multi-head attention (ReLU-gated projections) on 8 Trainium2 NeuronCores.

Problem (hardcoded): B=4, S=1024, H=1024, NH=16, DH=64.
  qp = relu(q @ Wq.T + bq); kp, vp likewise
  alpha = softmax(qh @ kh.T / sqrt(DH)) * mask[q]
  out = (alpha @ vh).reshape(B,S,H) + query

Sharding: 8 cores = 4 batches x 2 head-groups (8 heads / 512 hidden cols each).

fp8 design: all matmuls in fp8 e4m3 (TRN2 flavor: with-inf, max finite 240).
Weights pre-scaled by 32 on the host so their N(0, 1/32) values use e4m3's
normal range; the 32x factors ride through the linear pipeline (qp,kp,vp
all carry 32x) and are compensated in the exp scale (1/(8*32^2)) and a
final /32 on the host. exp also subtracts 3.0 (cancels in softmax) to
keep pt under the 240 cap. Projections and AV use MatmulPerfMode.DoubleRow
(K=256 per instruction, 2x PE throughput); alpha matmuls are
output-rate-bound so they stay plain fp8 with the kz zero-padded-K trick.
The AV stationary keeps a ones column (M=65) so row 64 accumulates sumexp
for free; the per-head V slot is padded to 68 bytes so DoubleRow weight
APs stay 4-byte aligned (ISA restriction s3_lw_dual_fp8).

Host pre-arranges x/w into the exact SBUF layouts so every input DMA is
128 partitions x 4KB contiguous. Consts load first so the PE-clock warmup
(and the ACT exp-table preload) start immediately.

Per-core device kernel (transposed "hidden-on-partitions" layout):
  stage 1: qpT[o,s], kpT[o,s] (transposed) and vp[s,o] (normal) projections
           with fused bias+relu, evacuated to fp8.
  stage 2: per head: alphaT[k,q] psum tiles; pt = exp(alpha*sc - 3) in fp8
           written into paired [128, 2048] tiles; AV via DoubleRow with the
           ones column -> unnormalized hidT (64,S) + sumexp (S) per head;
           host divides, applies mask, adds residual.
"""
import sys

sys.path.insert(0, "/opt/trn_rl_repo")

import os
import numpy as np
import ml_dtypes

import concourse.bass as bass
import concourse.tile as tile
from concourse import bacc, mybir
from concourse import bass_utils

B, S, H = 4, 1024, 1024
NH, DH = 16, 64
NCORES = 8
GROUPS = 2          # head-groups (tensor-parallel dim)
HL = NH // GROUPS   # heads per core = 8
GH = H // GROUPS    # hidden cols per core = 512
KT = H // 128       # contraction k-tiles = 8
OT = GH // 128      # output o-tiles per core = 4
SCALE = 1.0 / float(np.sqrt(DH))
ESC = 32.0          # fp8 weight pre-scale (TRN2 fp8e4 = e4m3-with-inf,
                    # max finite 240: keep relu'd projections under ~170)
VW8 = HL * 68       # padded per-head v slot (64 v + 1 ones + 3 pad) = 544

MODE = os.environ.get("BASS_MM_DT", "fp8")

F32 = mybir.dt.float32
BF16 = mybir.dt.bfloat16
FP8 = mybir.dt.float8e4
DR = mybir.MatmulPerfMode.DoubleRow
E4 = ml_dtypes.float8_e4m3   # e4m3 WITH inf (max 240) — matches TRN2 hw


def build_fp8():
    nc = bacc.Bacc("TRN2", target_bir_lowering=False, debug=False,
                   num_devices=NCORES)

    # x/w arrive pre-arranged in SBUF layout: [128, KT*512] per s-chunk
    x_d = {(w, sc): nc.dram_tensor(f"x{w}{sc}", [128, KT * 512], FP8,
                                   kind="ExternalInput").ap()
           for w in "qkv" for sc in range(2)}
    w_d = {w: nc.dram_tensor(f"w{w}", [128, KT * GH], FP8,
                             kind="ExternalInput").ap()
           for w in "qkv"}
    bqk_d = nc.dram_tensor("bqk", [128, 2 * OT], F32, kind="ExternalInput").ap()
    bv_d = nc.dram_tensor("bv", [1, GH], FP8, kind="ExternalInput").ap()
    ones_d = nc.dram_tensor("onesd", [128, 128], FP8,
                            kind="ExternalInput").ap()
    hid_d = nc.dram_tensor("hid", [HL * (DH + 1), S], F32,
                           kind="ExternalOutput").ap()

    EXP_SCALE = SCALE / (ESC * ESC)
    EXP_BIAS = -3.0   # pt = exp(alpha/8 - 3): keeps exp under e4m3 max 240;
                      # cancels in hid/sumexp

    with tile.TileContext(nc) as tc:
        with tc.tile_pool(name="sb", bufs=1) as sb, \
             tc.tile_pool(name="ps", bufs=1, space="PSUM") as ps:

            # ---- persistent tiles ----
            wq_t = sb.tile([128, KT * GH], FP8, tag="wq", name="wq")
            wk_t = sb.tile([128, KT * GH], FP8, tag="wk", name="wk")
            wv_t = sb.tile([128, KT * GH], FP8, tag="wv", name="wv")
            qp_t = [sb.tile([128, S], FP8, tag=f"qp{t}", name=f"qp{t}")
                    for t in range(OT)]
            kz_t = [[sb.tile([128, S], FP8, tag=f"kz{t}{h}",
                             name=f"kz{t}{h}") for h in range(2)]
                    for t in range(OT)]
            kz_zeroed = set()
            vp_t = sb.tile([128, KT * VW8], FP8, tag="vp", name="vp")
            bqk_t = sb.tile([128, 2 * OT], F32, tag="bqk", name="bqk")
            bv_t = sb.tile([1, GH], FP8, tag="bv", name="bv")
            ones_t = sb.tile([1, 128], FP8, tag="ones", name="ones")
            ones64_t = sb.tile([128, KT * HL], FP8, tag="ones64",
                               name="ones64")
            expb_t = sb.tile([128, 1], F32, tag="expb", name="expb")
            nc.vector.memset(expb_t[:], EXP_BIAS)

            # ---- warmup from memset tiles: no DMA dependency, so the PE
            #      clock ramp and the ACT exp-table preload start at ~1us ----
            wstat = sb.tile([1, 128], FP8, tag="wstat", name="wstat")
            wmov = sb.tile([1, 512], FP8, tag="wmov", name="wmov")
            nc.vector.memset(wstat[:], 1.0)
            nc.vector.memset(wmov[:], 1.0)
            warm = ps.tile([65, 512], F32, tag="av", bufs=2, name="warm")
            for i in range(8):
                nc.tensor.matmul(warm[:], wstat[:, 0:65], wmov[:],
                                 start=True, stop=True)
            dummy_exp = sb.tile([1, 8], F32, tag="dummy_exp", name="dummy_exp")
            nc.scalar.activation(dummy_exp[:], wmov[0:1, 0:8],
                                 mybir.ActivationFunctionType.Exp, scale=1.0)

            # ---- loads: whole tiles (4KB contiguous runs), three rings in
            #      parallel, priority-ordered by first use ----
            x_t = {}
            rings = [nc.sync, nc.scalar, nc.gpsimd]
            ring_i = [0]

            def x_ld(which, sc, eng):
                t = sb.tile([128, KT * 512], FP8, tag=f"x{which}{sc}",
                            name=f"x{which}_{sc}")
                x_t[(which, sc)] = t
                eng.dma_start(t[:], x_d[(which, sc)])

            def x3(which, sc):
                return x_t[(which, sc)][:].rearrange("p (k s) -> p k s", s=512)

            nc.sync.dma_start(bqk_t[:], bqk_d)
            nc.scalar.dma_start(bv_t[:], bv_d)
            nc.gpsimd.dma_start(ones_t[:], ones_d[0:1, :])
            nc.sync.dma_start(ones64_t[:], ones_d[:, 0:KT * HL])
            x_ld("q", 0, nc.sync)
            x_ld("k", 0, nc.scalar)
            nc.gpsimd.dma_start(wq_t[:], w_d["q"])
            nc.gpsimd.dma_start(wk_t[:], w_d["k"])
            x_ld("k", 1, nc.sync)
            x_ld("q", 1, nc.scalar)
            x_ld("v", 0, nc.sync)
            x_ld("v", 1, nc.scalar)
            nc.gpsimd.dma_start(wv_t[:], w_d["v"])

            # ones column of the AV stationary
            v4 = vp_t[:].rearrange("p (k n c) -> p k n c", n=HL, c=68)
            nc.vector.tensor_copy(
                v4[:, :, :, DH:DH + 1],
                ones64_t[:].rearrange("p (k n one) -> p k n one", n=HL, one=1))

            def proj_qk(sc, ot, which):
                """one o-tile, one s-chunk of the transposed q/k projection"""
                w_t = wq_t if which == "q" else wk_t
                pp = ps.tile([128, 1024], F32, tag="alpha", bufs=3,
                             name=f"pp{which}_{sc}_{ot}")
                w3 = w_t[:].rearrange("p (k o) -> p k o", o=GH)
                xv_ = x3(which, sc)
                for kp in range(KT // 2):
                    nc.tensor.matmul(
                        pp[:, 0:512],
                        w3[:, 2 * kp:2 * kp + 2, ot * 128:(ot + 1) * 128],
                        xv_[:, 2 * kp:2 * kp + 2, :],
                        start=(kp == 0), stop=(kp == KT // 2 - 1),
                        perf_mode=DR)
                wi = 0 if which == "q" else 1
                bias = bqk_t[:, wi * OT + ot:wi * OT + ot + 1]
                ssl = slice(sc * 512, (sc + 1) * 512)
                if which == "q":
                    nc.vector.tensor_scalar(
                        qp_t[ot][:, ssl], pp[:, 0:512], bias, 0.0,
                        mybir.AluOpType.add, mybir.AluOpType.max)
                else:
                    for h in range(2):
                        pr = slice(h * 64, h * 64 + 64)
                        nc.vector.tensor_scalar(
                            kz_t[ot][h][pr, ssl], pp[pr, 0:512], bias[pr, :],
                            0.0, mybir.AluOpType.add, mybir.AluOpType.max)

            def proj_v(sc, j):
                """one s-tile (128 rows of vp) within chunk sc"""
                st = sc * 4 + j
                pp = ps.tile([128, 1024], F32, tag="alpha", bufs=3,
                             name=f"ppv_{st}")
                nc.tensor.matmul(pp[:, 0:512], ones_t[:], bv_t[:],
                                 start=True, stop=False)
                wv3 = wv_t[:].rearrange("p (k o) -> p k o", o=GH)
                xv_ = x3("v", sc)
                for kp in range(KT // 2):
                    nc.tensor.matmul(
                        pp[:, 0:512],
                        xv_[:, 2 * kp:2 * kp + 2, j * 128:(j + 1) * 128],
                        wv3[:, 2 * kp:2 * kp + 2, :],
                        start=False, stop=(kp == KT // 2 - 1),
                        perf_mode=DR)
                v3 = vp_t[:, st * VW8:(st + 1) * VW8].rearrange(
                    "p (n c) -> p n c", c=68)
                p3 = pp[:, 0:512].rearrange("p (n c) -> p n c", c=DH)
                nc.vector.tensor_scalar(
                    v3[:, :, 0:DH], p3, 0.0, None, mybir.AluOpType.max)

            pt_all = {}

            def alphas(n0):
                """alpha + exp for head pair (n0, n0+1), head-major so each
                head's pt tiles complete early and its AV can start while the
                other head's exps still stream.  pt tiles are paired
                [128, 2048] (two k-tiles) so AV consumes them via DoubleRow."""
                t = n0 // 2
                if t not in kz_zeroed:
                    kz_zeroed.add(t)
                    nc.gpsimd.memset(kz_t[t][0][64:128, :], 0.0)
                    nc.gpsimd.memset(kz_t[t][1][0:64, :], 0.0)
                for h in range(2):
                    pts = []
                    cur = None
                    for k in range(KT):
                        apt = ps.tile([128, 1024], F32, tag="alpha", bufs=3,
                                      name=f"alp_{n0 + h}_{k}")
                        for qc in range(2):
                            nc.tensor.matmul(
                                apt[:, qc * 512:(qc + 1) * 512],
                                kz_t[t][h][:, k * 128:(k + 1) * 128],
                                qp_t[t][:, qc * 512:(qc + 1) * 512],
                                start=True, stop=True)
                        half = k % 2
                        if half == 0:
                            cur = sb.tile([128, 2048], FP8, tag="pt",
                                          bufs=16, name=f"pt_{n0 + h}_{k}")
                            pts.append(cur)
                        nc.scalar.activation(
                            cur[:, half * 1024:(half + 1) * 1024], apt[:],
                            mybir.ActivationFunctionType.Exp, scale=EXP_SCALE,
                            bias=expb_t[:])
                    pt_all[n0 + h] = pts

            def avs(n):
                pts = pt_all.pop(n)
                hid_t = sb.tile([DH + 1, S], F32, tag="hid",
                                bufs=3, name=f"hid_{n}")
                for qc in range(2):
                    av = ps.tile([DH + 1, 512], F32, tag="av", bufs=2,
                                 name=f"av_{n}_{qc}")
                    for kp in range(KT // 2):
                        nc.tensor.matmul(
                            av[:],
                            v4[:, 2 * kp:2 * kp + 2, n, 0:DH + 1],
                            pts[kp][:].rearrange(
                                "p (k s) -> p k s",
                                s=1024)[:, :, qc * 512:(qc + 1) * 512],
                            start=(kp == 0), stop=(kp == KT // 2 - 1),
                            perf_mode=DR)
                    nc.vector.tensor_copy(
                        hid_t[:, qc * 512:(qc + 1) * 512], av[:])
                    eng = rings[ring_i[0] % 3]
                    ring_i[0] += 1
                    eng.dma_start(
                        hid_d[n * (DH + 1):(n + 1) * (DH + 1),
                              qc * 512:(qc + 1) * 512],
                        hid_t[:, qc * 512:(qc + 1) * 512])

            # ---- emission schedule: keep the ACT exp stream (the
            #      co-bottleneck) hot from ~alphas(0) to the end ----
            proj_qk(0, 0, "q")
            proj_qk(0, 0, "k")
            proj_qk(1, 0, "q")
            proj_qk(1, 0, "k")
            alphas(0)
            proj_qk(0, 1, "q")
            proj_qk(0, 1, "k")
            proj_qk(1, 1, "q")
            proj_qk(1, 1, "k")
            proj_v(0, 0)
            proj_v(0, 1)
            alphas(2)
            proj_v(0, 2)
            proj_v(0, 3)
            for j in range(4):
                proj_v(1, j)
            avs(0)
            avs(1)
            proj_qk(0, 2, "q")
            proj_qk(0, 2, "k")
            proj_qk(1, 2, "q")
            proj_qk(1, 2, "k")
            alphas(4)
            avs(2)
            avs(3)
            proj_qk(0, 3, "q")
            proj_qk(0, 3, "k")
            proj_qk(1, 3, "q")
            proj_qk(1, 3, "k")
            alphas(6)
            avs(4)
            avs(5)
            avs(6)
            avs(7)

    nc.compile()
    return nc


_NC_CACHE = {}


def _get_nc(mode):
    if mode not in _NC_CACHE:
        if mode != "fp8":
            raise ValueError(f"unsupported mode {mode}")
        _NC_CACHE[mode] = build_fp8()
    return _NC_CACHE[mode]


def _sbuf_layout_x(xT):
    """[H, S] transposed input -> per-chunk [128, KT*512] SBUF image"""
    x4 = xT.reshape(KT, 128, 2, 512)          # [k, p, sc, s]
    return [np.ascontiguousarray(
        x4[:, :, sc, :].transpose(1, 0, 2).reshape(128, KT * 512)).astype(E4)
        for sc in range(2)]


def _sbuf_layout_w(wT):
    """[H, GH] transposed weight -> [128, KT*GH] SBUF image"""
    w3 = wT.reshape(KT, 128, GH)
    return np.ascontiguousarray(
        w3.transpose(1, 0, 2).reshape(128, KT * GH)).astype(E4)


def _prep_inputs(inputs):
    q = np.asarray(inputs["query"], np.float32)
    k = np.asarray(inputs["key"], np.float32)
    v = np.asarray(inputs["value"], np.float32)
    Wq = np.asarray(inputs["Wq"], np.float32)
    Wk = np.asarray(inputs["Wk"], np.float32)
    Wv = np.asarray(inputs["Wv"], np.float32)
    bq = np.asarray(inputs["bq"], np.float32)
    bk = np.asarray(inputs["bk"], np.float32)
    bv = np.asarray(inputs["bv"], np.float32)

    xq = [_sbuf_layout_x(q[b].T) for b in range(B)]
    xk = [_sbuf_layout_x(k[b].T) for b in range(B)]
    xv = [_sbuf_layout_x(v[b].T) for b in range(B)]
    in_maps = []
    for c in range(NCORES):
        b, g = c // GROUPS, c % GROUPS
        sl = slice(g * GH, (g + 1) * GH)
        bqk = np.stack([(ESC * bq[sl]).reshape(OT, 128).T,
                        (ESC * bk[sl]).reshape(OT, 128).T],
                       1).reshape(128, 2 * OT)
        in_maps.append({
            "xq0": xq[b][0], "xq1": xq[b][1],
            "xk0": xk[b][0], "xk1": xk[b][1],
            "xv0": xv[b][0], "xv1": xv[b][1],
            "wq": _sbuf_layout_w(ESC * Wq[sl, :].T),
            "wk": _sbuf_layout_w(ESC * Wk[sl, :].T),
            "wv": _sbuf_layout_w(ESC * Wv[sl, :].T),
            "bqk": np.ascontiguousarray(bqk, dtype=np.float32),
            "bv": np.ascontiguousarray(ESC * bv[None, sl]).astype(E4),
            "onesd": np.ones((128, 128), E4),
        })
    return in_maps


def run(inputs, mode=MODE, trace=False):
    nc = _get_nc(mode)
    in_maps = _prep_inputs(inputs)
    res = bass_utils.run_bass_kernel_spmd(
        nc, in_maps, core_ids=list(range(NCORES)), trace=trace)

    masks = np.asarray(inputs["masks"], np.float32)
    query = np.asarray(inputs["query"], np.float32)
    out = np.empty((B, S, H), np.float32)
    for c in range(NCORES):
        b, g = c // GROUPS, c % GROUPS
        hid = res.results[c]["hid"].reshape(HL, DH + 1, S)
        hT = hid[:, :DH, :]                      # (HL, DH, S)  (32x scaled)
        se = hid[:, DH, :]                       # (HL, S)
        blk = (hT / (ESC * se[:, None, :])).transpose(2, 0, 1).reshape(S, GH)
        out[b, :, g * GH:(g + 1) * GH] = blk
    out = out * masks[:, :, None] + query
    return out, res


def kernel(**inputs) -> np.ndarray:
    out, _ = run(inputs)
    return out


# revision 46
# speedup vs baseline: 1.4481x; 1.0419x over previous
"""Multi-head attention (ReLU-gated projections) on 8 Trainium2 NeuronCores.

Problem (hardcoded): B=4, S=1024, H=1024, NH=16, DH=64.
  qp = relu(q @ Wq.T + bq); kp, vp likewise
  alpha = softmax(qh @ kh.T / sqrt(DH)) * mask[q]
  out = (alpha @ vh).reshape(B,S,H) + query

Sharding: 8 cores = 4 batches x 2 head-groups (8 heads / 512 hidden cols each).

fp8 design: all matmuls in fp8 e4m3 (TRN2 flavor: with-inf, max finite 240).
Weights pre-scaled by 32 on the host so their N(0, 1/32) values use e4m3's
normal range; the 32x factors ride through the linear pipeline (qp,kp,vp
all carry 32x) and are compensated in the exp scale (1/(8*32^2)) and a
final /32 on the host. exp also subtracts 3.0 (cancels in softmax) to
keep pt under the 240 cap. Projections and AV use MatmulPerfMode.DoubleRow
(K=256 per instruction, 2x PE throughput); alpha matmuls are
output-rate-bound so they stay plain fp8 with the kz zero-padded-K trick.
The AV stationary keeps a ones column (M=65) so row 64 accumulates sumexp
for free; the per-head V slot is padded to 68 bytes so DoubleRow weight
APs stay 4-byte aligned (ISA restriction s3_lw_dual_fp8).

Host pre-arranges x/w into the exact SBUF layouts so every input DMA is
128 partitions x 4KB contiguous. Consts load first so the PE-clock warmup
(and the ACT exp-table preload) start immediately.

Per-core device kernel (transposed "hidden-on-partitions" layout):
  stage 1: qpT[o,s], kpT[o,s] (transposed) and vp[s,o] (normal) projections
           with fused bias+relu, evacuated to fp8.
  stage 2: per head: alphaT[k,q] psum tiles; pt = exp(alpha*sc - 3) in fp8
           written into paired [128, 2048] tiles; AV via DoubleRow with the
           ones column -> unnormalized hidT (64,S) + sumexp (S) per head;
           host divides, applies mask, adds residual.
"""
import sys

sys.path.insert(0, "/opt/trn_rl_repo")

import os
import numpy as np
import ml_dtypes

import concourse.bass as bass
import concourse.tile as tile
from concourse import bacc, mybir
from concourse import bass_utils

B, S, H = 4, 1024, 1024
NH, DH = 16, 64
NCORES = 8
GROUPS = 2          # head-groups (tensor-parallel dim)
HL = NH // GROUPS   # heads per core = 8
GH = H // GROUPS    # hidden cols per core = 512
KT = H // 128       # contraction k-tiles = 8
OT = GH // 128      # output o-tiles per core = 4
SCALE = 1.0 / float(np.sqrt(DH))
ESC = 32.0          # fp8 weight pre-scale (TRN2 fp8e4 = e4m3-with-inf,
                    # max finite 240: keep relu'd projections under ~170)
VW8 = HL * 68       # padded per-head v slot (64 v + 1 ones + 3 pad) = 544

MODE = os.environ.get("BASS_MM_DT", "fp8")

F32 = mybir.dt.float32
BF16 = mybir.dt.bfloat16
FP8 = mybir.dt.float8e4
DR = mybir.MatmulPerfMode.DoubleRow
E4 = ml_dtypes.float8_e4m3   # e4m3 WITH inf (max 240) — matches TRN2 hw


def build_fp8():
    nc = bacc.Bacc("TRN2", target_bir_lowering=False, debug=False,
                   num_devices=NCORES)

    # x/w arrive pre-arranged in SBUF layout: [128, KT*512] per s-chunk
    x_d = {(w, sc): nc.dram_tensor(f"x{w}{sc}", [128, KT * 512], FP8,
                                   kind="ExternalInput").ap()
           for w in "qkv" for sc in range(2)}
    w_d = {w: nc.dram_tensor(f"w{w}", [128, KT * GH], FP8,
                             kind="ExternalInput").ap()
           for w in "qkv"}
    bqk_d = nc.dram_tensor("bqk", [128, 2 * OT], F32, kind="ExternalInput").ap()
    bv_d = nc.dram_tensor("bv", [1, GH], FP8, kind="ExternalInput").ap()
    ones_d = nc.dram_tensor("onesd", [128, 128], FP8,
                            kind="ExternalInput").ap()
    hid_d = nc.dram_tensor("hid", [HL * (DH + 1), S], F32,
                           kind="ExternalOutput").ap()

    EXP_SCALE = SCALE / (ESC * ESC)
    EXP_BIAS = -3.0   # pt = exp(alpha/8 - 3): keeps exp under e4m3 max 240;
                      # cancels in hid/sumexp

    with tile.TileContext(nc) as tc:
        with tc.tile_pool(name="sb", bufs=1) as sb, \
             tc.tile_pool(name="ps", bufs=1, space="PSUM") as ps:

            # ---- persistent tiles ----
            wq_t = sb.tile([128, KT * GH], FP8, tag="wq", name="wq")
            wk_t = sb.tile([128, KT * GH], FP8, tag="wk", name="wk")
            wv_t = sb.tile([128, KT * GH], FP8, tag="wv", name="wv")
            qp_t = [sb.tile([128, S], FP8, tag=f"qp{t}", name=f"qp{t}")
                    for t in range(OT)]
            kz_t = [[sb.tile([128, S], FP8, tag=f"kz{t}{h}",
                             name=f"kz{t}{h}") for h in range(2)]
                    for t in range(OT)]
            kz_zeroed = set()
            vp_t = sb.tile([128, KT * VW8], FP8, tag="vp", name="vp")
            bqk_t = sb.tile([128, 2 * OT], F32, tag="bqk", name="bqk")
            bv_t = sb.tile([1, GH], FP8, tag="bv", name="bv")
            ones_t = sb.tile([1, 128], FP8, tag="ones", name="ones")
            ones64_t = sb.tile([128, KT * HL], FP8, tag="ones64",
                               name="ones64")
            expb_t = sb.tile([128, 1], F32, tag="expb", name="expb")
            nc.vector.memset(expb_t[:], EXP_BIAS)

            # ---- warmup from memset tiles: no DMA dependency, so the PE
            #      clock ramp and the ACT exp-table preload start at ~1us ----
            wstat = sb.tile([1, 128], FP8, tag="wstat", name="wstat")
            wmov = sb.tile([1, 512], FP8, tag="wmov", name="wmov")
            nc.vector.memset(wstat[:], 1.0)
            nc.vector.memset(wmov[:], 1.0)
            warm = ps.tile([65, 512], F32, tag="av", bufs=2, name="warm")
            for i in range(12):
                nc.tensor.matmul(warm[:], wstat[:, 0:65], wmov[:],
                                 start=True, stop=True)
            dummy_exp = sb.tile([1, 8], F32, tag="dummy_exp", name="dummy_exp")
            nc.scalar.activation(dummy_exp[:], wmov[0:1, 0:8],
                                 mybir.ActivationFunctionType.Exp, scale=1.0)

            # ---- loads: whole tiles (4KB contiguous runs), three rings in
            #      parallel, priority-ordered by first use ----
            x_t = {}
            rings = [nc.sync, nc.scalar, nc.gpsimd]
            ring_i = [0]

            def x_ld(which, sc, eng):
                t = sb.tile([128, KT * 512], FP8, tag=f"x{which}{sc}",
                            name=f"x{which}_{sc}")
                x_t[(which, sc)] = t
                eng.dma_start(t[:], x_d[(which, sc)])

            def x3(which, sc):
                return x_t[(which, sc)][:].rearrange("p (k s) -> p k s", s=512)

            nc.sync.dma_start(bqk_t[:], bqk_d)
            x_ld("q", 0, nc.sync)
            x_ld("k", 0, nc.scalar)
            nc.gpsimd.dma_start(wq_t[:], w_d["q"])
            nc.gpsimd.dma_start(wk_t[:], w_d["k"])
            x_ld("k", 1, nc.sync)
            x_ld("q", 1, nc.scalar)
            nc.gpsimd.dma_start(bv_t[:], bv_d)
            nc.gpsimd.dma_start(ones_t[:], ones_d[0:1, :])
            nc.sync.dma_start(ones64_t[:], ones_d[:, 0:KT * HL])
            x_ld("v", 0, nc.sync)
            x_ld("v", 1, nc.scalar)
            nc.gpsimd.dma_start(wv_t[:], w_d["v"])

            # ones column of the AV stationary
            v4 = vp_t[:].rearrange("p (k n c) -> p k n c", n=HL, c=68)
            nc.vector.tensor_copy(
                v4[:, :, :, DH:DH + 1],
                ones64_t[:].rearrange("p (k n one) -> p k n one", n=HL, one=1))

            pp_live = {}

            def proj_qk(sc, ot, which, part=None):
                """one o-tile, one s-chunk of the transposed q/k projection;
                part 0/1 emit half the DR chain each (fill-unit sizing),
                part None emits the whole group."""
                w_t = wq_t if which == "q" else wk_t
                w3 = w_t[:].rearrange("p (k o) -> p k o", o=GH)
                xv_ = x3(which, sc)
                if part != 1:
                    pp_live[(sc, ot, which)] = ps.tile(
                        [128, 1024], F32, tag="alpha", bufs=3,
                        name=f"pp{which}_{sc}_{ot}")
                pp = pp_live[(sc, ot, which)]
                kps = range(KT // 2) if part is None else (
                    range(2) if part == 0 else range(2, 4))
                for kp in kps:
                    nc.tensor.matmul(
                        pp[:, 0:512],
                        w3[:, 2 * kp:2 * kp + 2, ot * 128:(ot + 1) * 128],
                        xv_[:, 2 * kp:2 * kp + 2, :],
                        start=(kp == 0), stop=(kp == KT // 2 - 1),
                        perf_mode=DR)
                if part == 0:
                    return
                wi = 0 if which == "q" else 1
                bias = bqk_t[:, wi * OT + ot:wi * OT + ot + 1]
                ssl = slice(sc * 512, (sc + 1) * 512)
                if which == "q":
                    nc.vector.tensor_scalar(
                        qp_t[ot][:, ssl], pp[:, 0:512], bias, 0.0,
                        mybir.AluOpType.add, mybir.AluOpType.max)
                else:
                    for h in range(2):
                        pr = slice(h * 64, h * 64 + 64)
                        nc.vector.tensor_scalar(
                            kz_t[ot][h][pr, ssl], pp[pr, 0:512], bias[pr, :],
                            0.0, mybir.AluOpType.add, mybir.AluOpType.max)
                pp_live.pop((sc, ot, which))

            def proj_v(sc, j, part=None):
                """one s-tile (128 rows of vp) within chunk sc"""
                st = sc * 4 + j
                wv3 = wv_t[:].rearrange("p (k o) -> p k o", o=GH)
                xv_ = x3("v", sc)
                if part != 1:
                    pp_live[("v", st)] = ps.tile([128, 1024], F32,
                                                 tag="alpha", bufs=3,
                                                 name=f"ppv_{st}")
                    nc.tensor.matmul(pp_live[("v", st)][:, 0:512],
                                     ones_t[:], bv_t[:],
                                     start=True, stop=False)
                pp = pp_live[("v", st)]
                kps = range(KT // 2) if part is None else (
                    range(2) if part == 0 else range(2, 4))
                for kp in kps:
                    nc.tensor.matmul(
                        pp[:, 0:512],
                        xv_[:, 2 * kp:2 * kp + 2, j * 128:(j + 1) * 128],
                        wv3[:, 2 * kp:2 * kp + 2, :],
                        start=False, stop=(kp == KT // 2 - 1),
                        perf_mode=DR)
                if part == 0:
                    return
                v3 = vp_t[:, st * VW8:(st + 1) * VW8].rearrange(
                    "p (n c) -> p n c", c=68)
                p3 = pp[:, 0:512].rearrange("p (n c) -> p n c", c=DH)
                nc.vector.tensor_scalar(
                    v3[:, :, 0:DH], p3, 0.0, None, mybir.AluOpType.max)
                pp_live.pop(("v", st))

            pt_all = {}
            fill_q = []

            def alphas(n0, pops=(1, 3, 5, 7)):
                """alpha + exp for head pair (n0, n0+1), head-major so each
                head's pt tiles complete early and its AV can start while the
                other head's exps still stream.  pt tiles are paired
                [128, 2048] (two k-tiles) so AV consumes them via DoubleRow.
                Between apt tiles, pop small PE work units from fill_q so the
                PE's ACT-rate-limited stall time does useful work."""
                t = n0 // 2
                if t not in kz_zeroed:
                    kz_zeroed.add(t)
                    nc.gpsimd.memset(kz_t[t][0][64:128, :], 0.0)
                    nc.gpsimd.memset(kz_t[t][1][0:64, :], 0.0)
                for h in range(2):
                    pts = []
                    cur = None
                    for k in range(KT):
                        apt = ps.tile([128, 1024], F32, tag="alpha", bufs=3,
                                      name=f"alp_{n0 + h}_{k}")
                        for qc in range(2):
                            nc.tensor.matmul(
                                apt[:, qc * 512:(qc + 1) * 512],
                                kz_t[t][h][:, k * 128:(k + 1) * 128],
                                qp_t[t][:, qc * 512:(qc + 1) * 512],
                                start=True, stop=True)
                        half = k % 2
                        if half == 0:
                            cur = sb.tile([128, 2048], FP8, tag="pt",
                                          bufs=32, name=f"pt_{n0 + h}_{k}")
                            pts.append(cur)
                        nc.scalar.activation(
                            cur[:, half * 1024:(half + 1) * 1024], apt[:],
                            mybir.ActivationFunctionType.Exp, scale=EXP_SCALE,
                            bias=expb_t[:])
                        if k in pops and fill_q:
                            fill_q.pop(0)()
                    pt_all[n0 + h] = pts

            hid_tiles = {}
            av_live = {}

            def avs_qc(n, qc, last=False, part=None):
                pts = pt_all[n]
                if qc == 0 and part != 1:
                    hid_tiles[n] = sb.tile([DH + 1, S], F32, tag="hid",
                                           bufs=3, name=f"hid_{n}")
                hid_t = hid_tiles[n]
                if part != 1:
                    av_live[(n, qc)] = ps.tile([DH + 1, 512], F32, tag="av",
                                               bufs=2, name=f"av_{n}_{qc}")
                av = av_live[(n, qc)]
                kps = range(KT // 2) if part is None else (
                    range(2) if part == 0 else range(2, 4))
                for kp in kps:
                    nc.tensor.matmul(
                        av[:],
                        v4[:, 2 * kp:2 * kp + 2, n, 0:DH + 1],
                        pts[kp][:].rearrange(
                            "p (k s) -> p k s",
                            s=1024)[:, :, qc * 512:(qc + 1) * 512],
                        start=(kp == 0), stop=(kp == KT // 2 - 1),
                        perf_mode=DR)
                if part == 0:
                    return
                av_live.pop((n, qc))
                if last:
                    # ACT is idle after its final exp — use it so the two
                    # tail evacuations run on different engines
                    nc.scalar.copy(
                        hid_t[:, qc * 512:(qc + 1) * 512], av[:])
                else:
                    nc.vector.tensor_copy(
                        hid_t[:, qc * 512:(qc + 1) * 512], av[:])
                # never the scalar ring: a DMA descriptor op there would
                # steal ~0.8us from the ACT exp stream
                eng = nc.sync if ring_i[0] % 2 == 0 else nc.gpsimd
                ring_i[0] += 1
                eng.dma_start(
                    hid_d[n * (DH + 1):(n + 1) * (DH + 1),
                          qc * 512:(qc + 1) * 512],
                    hid_t[:, qc * 512:(qc + 1) * 512])
                if qc == 1:
                    pt_all.pop(n)
                    hid_tiles.pop(n)

            # ---- emission schedule: the exp stream (ACT) is the metronome.
            #      All other PE work is queued as fill units popped between
            #      alpha tiles, so the PE's ACT-limited stalls do the
            #      projections and AV chunks. Queue order respects deps:
            #      o-tile t's projections drain inside alphas(2(t-1)). ----
            def u2(f, *a):
                fill_q.append(lambda: f(*a, part=0))
                fill_q.append(lambda: f(*a, part=1))

            # alphas(0) k-tiles 0-3 read only the sc0 half of kz o-tile 0, so
            # the sc1 k-projection rides the fill queue (its parts pop at
            # k=1,2 — done before the k=4 alpha tile needs it)
            proj_qk(0, 0, "q")
            proj_qk(0, 0, "k")
            proj_qk(1, 0, "q")
            u2(proj_qk, 1, 0, "k")
            u2(proj_qk, 0, 1, "q")
            u2(proj_qk, 0, 1, "k")
            u2(proj_qk, 1, 1, "q")
            u2(proj_qk, 1, 1, "k")
            u2(proj_v, 0, 0)
            u2(proj_v, 0, 1)
            alphas(0, pops=(1, 2, 3, 4, 5, 6, 7))
            u2(proj_qk, 0, 2, "q")
            u2(proj_qk, 0, 2, "k")
            u2(proj_qk, 1, 2, "q")
            u2(proj_qk, 1, 2, "k")
            u2(proj_v, 0, 2)
            u2(proj_v, 0, 3)
            u2(proj_v, 1, 0)
            u2(proj_v, 1, 1)
            alphas(2, pops=tuple(range(KT)))
            u2(proj_qk, 0, 3, "q")
            u2(proj_qk, 0, 3, "k")
            u2(proj_qk, 1, 3, "q")
            u2(proj_qk, 1, 3, "k")
            u2(proj_v, 1, 2)
            u2(proj_v, 1, 3)
            u2(avs_qc, 0, 0)
            u2(avs_qc, 0, 1)
            alphas(4, pops=tuple(range(KT)))
            u2(avs_qc, 1, 0)
            u2(avs_qc, 1, 1)
            alphas(6, pops=(2, 3, 4, 5))
            while fill_q:
                fill_q.pop(0)()
            avs_qc(2, 0)
            avs_qc(2, 1)
            avs_qc(3, 0)
            avs_qc(3, 1)
            avs_qc(4, 0)
            avs_qc(4, 1)
            avs_qc(5, 0)
            avs_qc(5, 1)
            avs_qc(6, 0)
            avs_qc(6, 1)
            avs_qc(7, 0)
            avs_qc(7, 1, last=True)

    nc.compile()
    return nc


_NC_CACHE = {}


def _get_nc(mode):
    if mode not in _NC_CACHE:
        if mode != "fp8":
            raise ValueError(f"unsupported mode {mode}")
        _NC_CACHE[mode] = build_fp8()
    return _NC_CACHE[mode]


def _sbuf_layout_x(xT):
    """[H, S] transposed input -> per-chunk [128, KT*512] SBUF image"""
    x4 = xT.reshape(KT, 128, 2, 512)          # [k, p, sc, s]
    return [np.ascontiguousarray(
        x4[:, :, sc, :].transpose(1, 0, 2).reshape(128, KT * 512)).astype(E4)
        for sc in range(2)]


def _sbuf_layout_w(wT):
    """[H, GH] transposed weight -> [128, KT*GH] SBUF image"""
    w3 = wT.reshape(KT, 128, GH)
    return np.ascontiguousarray(
        w3.transpose(1, 0, 2).reshape(128, KT * GH)).astype(E4)


def _prep_inputs(inputs):
    q = np.asarray(inputs["query"], np.float32)
    k = np.asarray(inputs["key"], np.float32)
    v = np.asarray(inputs["value"], np.float32)
    Wq = np.asarray(inputs["Wq"], np.float32)
    Wk = np.asarray(inputs["Wk"], np.float32)
    Wv = np.asarray(inputs["Wv"], np.float32)
    bq = np.asarray(inputs["bq"], np.float32)
    bk = np.asarray(inputs["bk"], np.float32)
    bv = np.asarray(inputs["bv"], np.float32)

    xq = [_sbuf_layout_x(q[b].T) for b in range(B)]
    xk = [_sbuf_layout_x(k[b].T) for b in range(B)]
    xv = [_sbuf_layout_x(v[b].T) for b in range(B)]
    in_maps = []
    for c in range(NCORES):
        b, g = c // GROUPS, c % GROUPS
        sl = slice(g * GH, (g + 1) * GH)
        bqk = np.stack([(ESC * bq[sl]).reshape(OT, 128).T,
                        (ESC * bk[sl]).reshape(OT, 128).T],
                       1).reshape(128, 2 * OT)
        in_maps.append({
            "xq0": xq[b][0], "xq1": xq[b][1],
            "xk0": xk[b][0], "xk1": xk[b][1],
            "xv0": xv[b][0], "xv1": xv[b][1],
            "wq": _sbuf_layout_w(ESC * Wq[sl, :].T),
            "wk": _sbuf_layout_w(ESC * Wk[sl, :].T),
            "wv": _sbuf_layout_w(ESC * Wv[sl, :].T),
            "bqk": np.ascontiguousarray(bqk, dtype=np.float32),
            "bv": np.ascontiguousarray(ESC * bv[None, sl]).astype(E4),
            "onesd": np.ones((128, 128), E4),
        })
    return in_maps


def run(inputs, mode=MODE, trace=False):
    nc = _get_nc(mode)
    in_maps = _prep_inputs(inputs)
    res = bass_utils.run_bass_kernel_spmd(
        nc, in_maps, core_ids=list(range(NCORES)), trace=trace)

    masks = np.asarray(inputs["masks"], np.float32)
    query = np.asarray(inputs["query"], np.float32)
    out = np.empty((B, S, H), np.float32)
    for c in range(NCORES):
        b, g = c // GROUPS, c % GROUPS
        hid = res.results[c]["hid"].reshape(HL, DH + 1, S)
        hT = hid[:, :DH, :]                      # (HL, DH, S)  (32x scaled)
        se = hid[:, DH, :]                       # (HL, S)
        blk = (hT / (ESC * se[:, None, :])).transpose(2, 0, 1).reshape(S, GH)
        out[b, :, g * GH:(g + 1) * GH] = blk
    out = out * masks[:, :, None] + query
    return out, res


def kernel(**inputs) -> np.ndarray:
    out, _ = run(inputs)
    return out


# revision 47
# speedup vs baseline: 1.4599x; 1.0081x over previous
"""Multi-head attention (ReLU-gated projections) on 8 Trainium2 NeuronCores.

Problem (hardcoded): B=4, S=1024, H=1024, NH=16, DH=64.
  qp = relu(q @ Wq.T + bq); kp, vp likewise
  alpha = softmax(qh @ kh.T / sqrt(DH)) * mask[q]
  out = (alpha @ vh).reshape(B,S,H) + query

Sharding: 8 cores = 4 batches x 2 head-groups (8 heads / 512 hidden cols each).

fp8 design: all matmuls in fp8 e4m3 (TRN2 flavor: with-inf, max finite 240).
Weights pre-scaled by 32 on the host so their N(0, 1/32) values use e4m3's
normal range; the 32x factors ride through the linear pipeline (qp,kp,vp
all carry 32x) and are compensated in the exp scale (1/(8*32^2)) and a
final /32 on the host. exp also subtracts 3.0 (cancels in softmax) to
keep pt under the 240 cap. Projections and AV use MatmulPerfMode.DoubleRow
(K=256 per instruction, 2x PE throughput); alpha matmuls are
output-rate-bound so they stay plain fp8 with the kz zero-padded-K trick.
The AV stationary keeps a ones column (M=65) so row 64 accumulates sumexp
for free; the per-head V slot is padded to 68 bytes so DoubleRow weight
APs stay 4-byte aligned (ISA restriction s3_lw_dual_fp8).

Host pre-arranges x/w into the exact SBUF layouts so every input DMA is
128 partitions x 4KB contiguous. Consts load first so the PE-clock warmup
(and the ACT exp-table preload) start immediately.

Per-core device kernel (transposed "hidden-on-partitions" layout):
  stage 1: qpT[o,s], kpT[o,s] (transposed) and vp[s,o] (normal) projections
           with fused bias+relu, evacuated to fp8.
  stage 2: per head: alphaT[k,q] psum tiles; pt = exp(alpha*sc - 3) in fp8
           written into paired [128, 2048] tiles; AV via DoubleRow with the
           ones column -> unnormalized hidT (64,S) + sumexp (S) per head;
           host divides, applies mask, adds residual.
"""
import sys

sys.path.insert(0, "/opt/trn_rl_repo")

import os
import numpy as np
import ml_dtypes

import concourse.bass as bass
import concourse.tile as tile
from concourse import bacc, mybir
from concourse import bass_utils

B, S, H = 4, 1024, 1024
NH, DH = 16, 64
NCORES = 8
GROUPS = 2          # head-groups (tensor-parallel dim)
HL = NH // GROUPS   # heads per core = 8
GH = H // GROUPS    # hidden cols per core = 512
KT = H // 128       # contraction k-tiles = 8
OT = GH // 128      # output o-tiles per core = 4
SCALE = 1.0 / float(np.sqrt(DH))
ESC = 32.0          # fp8 weight pre-scale (TRN2 fp8e4 = e4m3-with-inf,
                    # max finite 240: keep relu'd projections under ~170)
VW8 = HL * 68       # padded per-head v slot (64 v + 1 ones + 3 pad) = 544

MODE = os.environ.get("BASS_MM_DT", "fp8")

F32 = mybir.dt.float32
BF16 = mybir.dt.bfloat16
FP8 = mybir.dt.float8e4
DR = mybir.MatmulPerfMode.DoubleRow
E4 = ml_dtypes.float8_e4m3   # e4m3 WITH inf (max 240) — matches TRN2 hw


def build_fp8():
    nc = bacc.Bacc("TRN2", target_bir_lowering=False, debug=False,
                   num_devices=NCORES)

    # x/w arrive pre-arranged in SBUF layout: [128, KT*512] per s-chunk
    x_d = {(w, sc): nc.dram_tensor(f"x{w}{sc}", [128, KT * 512], FP8,
                                   kind="ExternalInput").ap()
           for w in "qkv" for sc in range(2)}
    w_d = {w: nc.dram_tensor(f"w{w}", [128, KT * GH], FP8,
                             kind="ExternalInput").ap()
           for w in "qkv"}
    bqk_d = nc.dram_tensor("bqk", [128, 2 * OT], F32, kind="ExternalInput").ap()
    bv_d = nc.dram_tensor("bv", [1, GH], FP8, kind="ExternalInput").ap()
    ones_d = nc.dram_tensor("onesd", [128, 128], FP8,
                            kind="ExternalInput").ap()
    hid_d = nc.dram_tensor("hid", [HL * (DH + 1), S], F32,
                           kind="ExternalOutput").ap()

    EXP_SCALE = SCALE / (ESC * ESC)
    EXP_BIAS = -3.0   # pt = exp(alpha/8 - 3): keeps exp under e4m3 max 240;
                      # cancels in hid/sumexp

    with tile.TileContext(nc) as tc:
        with tc.tile_pool(name="sb", bufs=1) as sb, \
             tc.tile_pool(name="ps", bufs=1, space="PSUM") as ps:

            # ---- persistent tiles ----
            wq_t = sb.tile([128, KT * GH], FP8, tag="wq", name="wq")
            wk_t = sb.tile([128, KT * GH], FP8, tag="wk", name="wk")
            wv_t = sb.tile([128, KT * GH], FP8, tag="wv", name="wv")
            qp_t = [sb.tile([128, S], FP8, tag=f"qp{t}", name=f"qp{t}")
                    for t in range(OT)]
            kz_t = [[sb.tile([128, S], FP8, tag=f"kz{t}{h}",
                             name=f"kz{t}{h}") for h in range(2)]
                    for t in range(OT)]
            kz_zeroed = set()
            vp_t = sb.tile([128, KT * VW8], FP8, tag="vp", name="vp")
            bqk_t = sb.tile([128, 2 * OT], F32, tag="bqk", name="bqk")
            bv_t = sb.tile([1, GH], FP8, tag="bv", name="bv")
            ones_t = sb.tile([1, 128], FP8, tag="ones", name="ones")
            ones64_t = sb.tile([128, KT * HL], FP8, tag="ones64",
                               name="ones64")
            expb_t = sb.tile([128, 1], F32, tag="expb", name="expb")
            nc.vector.memset(expb_t[:], EXP_BIAS)

            # ---- warmup from memset tiles: no DMA dependency, so the PE
            #      clock ramp and the ACT exp-table preload start at ~1us ----
            wstat = sb.tile([1, 128], FP8, tag="wstat", name="wstat")
            wmov = sb.tile([1, 512], FP8, tag="wmov", name="wmov")
            nc.vector.memset(wstat[:], 1.0)
            nc.vector.memset(wmov[:], 1.0)
            warm = ps.tile([65, 512], F32, tag="av", bufs=2, name="warm")
            for i in range(12):
                nc.tensor.matmul(warm[:], wstat[:, 0:65], wmov[:],
                                 start=True, stop=True)
            dummy_exp = sb.tile([1, 8], F32, tag="dummy_exp", name="dummy_exp")
            nc.scalar.activation(dummy_exp[:], wmov[0:1, 0:8],
                                 mybir.ActivationFunctionType.Exp, scale=1.0)

            # ---- loads: whole tiles (4KB contiguous runs), three rings in
            #      parallel, priority-ordered by first use ----
            x_t = {}
            rings = [nc.sync, nc.scalar, nc.gpsimd]
            ring_i = [0]

            def x_ld(which, sc, eng):
                t = sb.tile([128, KT * 512], FP8, tag=f"x{which}{sc}",
                            name=f"x{which}_{sc}")
                x_t[(which, sc)] = t
                eng.dma_start(t[:], x_d[(which, sc)])

            def x3(which, sc):
                return x_t[(which, sc)][:].rearrange("p (k s) -> p k s", s=512)

            nc.sync.dma_start(bqk_t[:], bqk_d)
            x_ld("q", 0, nc.sync)
            x_ld("k", 0, nc.scalar)
            nc.gpsimd.dma_start(wq_t[:], w_d["q"])
            nc.gpsimd.dma_start(wk_t[:], w_d["k"])
            x_ld("k", 1, nc.sync)
            x_ld("q", 1, nc.scalar)
            nc.gpsimd.dma_start(bv_t[:], bv_d)
            nc.gpsimd.dma_start(ones_t[:], ones_d[0:1, :])
            nc.sync.dma_start(ones64_t[:], ones_d[:, 0:KT * HL])
            x_ld("v", 0, nc.sync)
            x_ld("v", 1, nc.scalar)
            nc.gpsimd.dma_start(wv_t[:], w_d["v"])

            # ones column of the AV stationary
            v4 = vp_t[:].rearrange("p (k n c) -> p k n c", n=HL, c=68)
            nc.vector.tensor_copy(
                v4[:, :, :, DH:DH + 1],
                ones64_t[:].rearrange("p (k n one) -> p k n one", n=HL, one=1))

            pp_live = {}

            def proj_qk(sc, ot, which, part=None):
                """one o-tile, one s-chunk of the transposed q/k projection;
                part 0/1 emit half the DR chain each (fill-unit sizing),
                part None emits the whole group."""
                w_t = wq_t if which == "q" else wk_t
                w3 = w_t[:].rearrange("p (k o) -> p k o", o=GH)
                xv_ = x3(which, sc)
                if part != 1:
                    pp_live[(sc, ot, which)] = ps.tile(
                        [128, 1024], F32, tag="alpha", bufs=3,
                        name=f"pp{which}_{sc}_{ot}")
                pp = pp_live[(sc, ot, which)]
                kps = range(KT // 2) if part is None else (
                    range(2) if part == 0 else range(2, 4))
                for kp in kps:
                    nc.tensor.matmul(
                        pp[:, 0:512],
                        w3[:, 2 * kp:2 * kp + 2, ot * 128:(ot + 1) * 128],
                        xv_[:, 2 * kp:2 * kp + 2, :],
                        start=(kp == 0), stop=(kp == KT // 2 - 1),
                        perf_mode=DR)
                if part == 0:
                    return
                wi = 0 if which == "q" else 1
                bias = bqk_t[:, wi * OT + ot:wi * OT + ot + 1]
                ssl = slice(sc * 512, (sc + 1) * 512)
                if which == "q":
                    nc.vector.tensor_scalar(
                        qp_t[ot][:, ssl], pp[:, 0:512], bias, 0.0,
                        mybir.AluOpType.add, mybir.AluOpType.max)
                else:
                    for h in range(2):
                        pr = slice(h * 64, h * 64 + 64)
                        nc.vector.tensor_scalar(
                            kz_t[ot][h][pr, ssl], pp[pr, 0:512], bias[pr, :],
                            0.0, mybir.AluOpType.add, mybir.AluOpType.max)
                pp_live.pop((sc, ot, which))

            def proj_v(sc, j, part=None):
                """one s-tile (128 rows of vp) within chunk sc"""
                st = sc * 4 + j
                wv3 = wv_t[:].rearrange("p (k o) -> p k o", o=GH)
                xv_ = x3("v", sc)
                if part != 1:
                    pp_live[("v", st)] = ps.tile([128, 1024], F32,
                                                 tag="alpha", bufs=3,
                                                 name=f"ppv_{st}")
                    nc.tensor.matmul(pp_live[("v", st)][:, 0:512],
                                     ones_t[:], bv_t[:],
                                     start=True, stop=False)
                pp = pp_live[("v", st)]
                kps = range(KT // 2) if part is None else (
                    range(2) if part == 0 else range(2, 4))
                for kp in kps:
                    nc.tensor.matmul(
                        pp[:, 0:512],
                        xv_[:, 2 * kp:2 * kp + 2, j * 128:(j + 1) * 128],
                        wv3[:, 2 * kp:2 * kp + 2, :],
                        start=False, stop=(kp == KT // 2 - 1),
                        perf_mode=DR)
                if part == 0:
                    return
                v3 = vp_t[:, st * VW8:(st + 1) * VW8].rearrange(
                    "p (n c) -> p n c", c=68)
                p3 = pp[:, 0:512].rearrange("p (n c) -> p n c", c=DH)
                nc.vector.tensor_scalar(
                    v3[:, :, 0:DH], p3, 0.0, None, mybir.AluOpType.max)
                pp_live.pop(("v", st))

            pt_all = {}
            fill_q = []

            def alphas(n0, pops=(1, 3, 5, 7)):
                """alpha + exp for head pair (n0, n0+1), head-major so each
                head's pt tiles complete early and its AV can start while the
                other head's exps still stream.  pt tiles are paired
                [128, 2048] (two k-tiles) so AV consumes them via DoubleRow.
                Between apt tiles, pop small PE work units from fill_q so the
                PE's ACT-rate-limited stall time does useful work."""
                t = n0 // 2
                if t not in kz_zeroed:
                    kz_zeroed.add(t)
                    nc.gpsimd.memset(kz_t[t][0][64:128, :], 0.0)
                    nc.gpsimd.memset(kz_t[t][1][0:64, :], 0.0)
                for h in range(2):
                    pts = []
                    cur = None
                    for k in range(KT):
                        apt = ps.tile([128, 1024], F32, tag="alpha", bufs=3,
                                      name=f"alp_{n0 + h}_{k}")
                        for qc in range(2):
                            nc.tensor.matmul(
                                apt[:, qc * 512:(qc + 1) * 512],
                                kz_t[t][h][:, k * 128:(k + 1) * 128],
                                qp_t[t][:, qc * 512:(qc + 1) * 512],
                                start=True, stop=True)
                        half = k % 2
                        if half == 0:
                            cur = sb.tile([128, 2048], FP8, tag="pt",
                                          bufs=32, name=f"pt_{n0 + h}_{k}")
                            pts.append(cur)
                        nc.scalar.activation(
                            cur[:, half * 1024:(half + 1) * 1024], apt[:],
                            mybir.ActivationFunctionType.Exp, scale=EXP_SCALE,
                            bias=expb_t[:])
                        if k in pops and fill_q:
                            fill_q.pop(0)()
                    pt_all[n0 + h] = pts

            hid_tiles = {}
            av_live = {}

            def avs_qc(n, qc, last=False, part=None):
                pts = pt_all[n]
                if qc == 0 and part != 1:
                    hid_tiles[n] = sb.tile([DH + 1, S], F32, tag="hid",
                                           bufs=3, name=f"hid_{n}")
                hid_t = hid_tiles[n]
                if part != 1:
                    av_live[(n, qc)] = ps.tile([DH + 1, 512], F32, tag="av",
                                               bufs=2, name=f"av_{n}_{qc}")
                av = av_live[(n, qc)]
                kps = range(KT // 2) if part is None else (
                    range(2) if part == 0 else range(2, 4))
                for kp in kps:
                    nc.tensor.matmul(
                        av[:],
                        v4[:, 2 * kp:2 * kp + 2, n, 0:DH + 1],
                        pts[kp][:].rearrange(
                            "p (k s) -> p k s",
                            s=1024)[:, :, qc * 512:(qc + 1) * 512],
                        start=(kp == 0), stop=(kp == KT // 2 - 1),
                        perf_mode=DR)
                if part == 0:
                    return
                av_live.pop((n, qc))
                if last:
                    # ACT is idle after its final exp — use it so the two
                    # tail evacuations run on different engines
                    nc.scalar.copy(
                        hid_t[:, qc * 512:(qc + 1) * 512], av[:])
                else:
                    nc.vector.tensor_copy(
                        hid_t[:, qc * 512:(qc + 1) * 512], av[:])
                # never the scalar ring: a DMA descriptor op there would
                # steal ~0.8us from the ACT exp stream
                eng = nc.sync if ring_i[0] % 2 == 0 else nc.gpsimd
                ring_i[0] += 1
                eng.dma_start(
                    hid_d[n * (DH + 1):(n + 1) * (DH + 1),
                          qc * 512:(qc + 1) * 512],
                    hid_t[:, qc * 512:(qc + 1) * 512])
                if qc == 1:
                    pt_all.pop(n)
                    hid_tiles.pop(n)

            # ---- emission schedule: the exp stream (ACT) is the metronome.
            #      All other PE work is queued as fill units popped between
            #      alpha tiles, so the PE's ACT-limited stalls do the
            #      projections and AV chunks. Queue order respects deps:
            #      o-tile t's projections drain inside alphas(2(t-1)). ----
            def u2(f, *a):
                fill_q.append(lambda: f(*a, part=0))
                fill_q.append(lambda: f(*a, part=1))

            proj_qk(0, 0, "q")
            proj_qk(0, 0, "k")
            proj_qk(1, 0, "q")
            proj_qk(1, 0, "k")
            u2(proj_qk, 0, 1, "q")
            u2(proj_qk, 0, 1, "k")
            u2(proj_qk, 1, 1, "q")
            u2(proj_qk, 1, 1, "k")
            u2(proj_v, 0, 0)
            u2(proj_v, 0, 1)
            alphas(0, pops=(1, 2, 3, 4, 5, 6, 7))
            u2(proj_qk, 0, 2, "q")
            u2(proj_qk, 0, 2, "k")
            u2(proj_qk, 1, 2, "q")
            u2(proj_qk, 1, 2, "k")
            u2(proj_v, 0, 2)
            u2(proj_v, 0, 3)
            u2(proj_v, 1, 0)
            u2(proj_v, 1, 1)
            alphas(2, pops=tuple(range(KT)))
            u2(proj_qk, 0, 3, "q")
            u2(proj_qk, 0, 3, "k")
            u2(proj_qk, 1, 3, "q")
            u2(proj_qk, 1, 3, "k")
            u2(proj_v, 1, 2)
            u2(proj_v, 1, 3)
            u2(avs_qc, 0, 0)
            u2(avs_qc, 0, 1)
            alphas(4, pops=tuple(range(KT)))
            u2(avs_qc, 1, 0)
            u2(avs_qc, 1, 1)
            alphas(6, pops=(2, 3, 4, 5))
            while fill_q:
                fill_q.pop(0)()
            avs_qc(2, 0)
            avs_qc(2, 1)
            avs_qc(3, 0)
            avs_qc(3, 1)
            avs_qc(4, 0)
            avs_qc(4, 1)
            avs_qc(5, 0)
            avs_qc(5, 1)
            avs_qc(6, 0)
            avs_qc(6, 1)
            avs_qc(7, 0)
            avs_qc(7, 1, last=True)

    nc.compile()
    return nc


_NC_CACHE = {}


def _get_nc(mode):
    if mode not in _NC_CACHE:
        if mode != "fp8":
            raise ValueError(f"unsupported mode {mode}")
        _NC_CACHE[mode] = build_fp8()
    return _NC_CACHE[mode]


def _sbuf_layout_x(xT):
    """[H, S] transposed input -> per-chunk [128, KT*512] SBUF image"""
    x4 = xT.reshape(KT, 128, 2, 512)          # [k, p, sc, s]
    return [np.ascontiguousarray(
        x4[:, :, sc, :].transpose(1, 0, 2).reshape(128, KT * 512)).astype(E4)
        for sc in range(2)]


def _sbuf_layout_w(wT):
    """[H, GH] transposed weight -> [128, KT*GH] SBUF image"""
    w3 = wT.reshape(KT, 128, GH)
    return np.ascontiguousarray(
        w3.transpose(1, 0, 2).reshape(128, KT * GH)).astype(E4)


def _prep_inputs(inputs):
    q = np.asarray(inputs["query"], np.float32)
    k = np.asarray(inputs["key"], np.float32)
    v = np.asarray(inputs["value"], np.float32)
    Wq = np.asarray(inputs["Wq"], np.float32)
    Wk = np.asarray(inputs["Wk"], np.float32)
    Wv = np.asarray(inputs["Wv"], np.float32)
    bq = np.asarray(inputs["bq"], np.float32)
    bk = np.asarray(inputs["bk"], np.float32)
    bv = np.asarray(inputs["bv"], np.float32)

    xq = [_sbuf_layout_x(q[b].T) for b in range(B)]
    xk = [_sbuf_layout_x(k[b].T) for b in range(B)]
    xv = [_sbuf_layout_x(v[b].T) for b in range(B)]
    in_maps = []
    for c in range(NCORES):
        b, g = c // GROUPS, c % GROUPS
        sl = slice(g * GH, (g + 1) * GH)
        bqk = np.stack([(ESC * bq[sl]).reshape(OT, 128).T,
                        (ESC * bk[sl]).reshape(OT, 128).T],
                       1).reshape(128, 2 * OT)
        in_maps.append({
            "xq0": xq[b][0], "xq1": xq[b][1],
            "xk0": xk[b][0], "xk1": xk[b][1],
            "xv0": xv[b][0], "xv1": xv[b][1],
            "wq": _sbuf_layout_w(ESC * Wq[sl, :].T),
            "wk": _sbuf_layout_w(ESC * Wk[sl, :].T),
            "wv": _sbuf_layout_w(ESC * Wv[sl, :].T),
            "bqk": np.ascontiguousarray(bqk, dtype=np.float32),
            "bv": np.ascontiguousarray(ESC * bv[None, sl]).astype(E4),
            "onesd": np.ones((128, 128), E4),
        })
    return in_maps


def run(inputs, mode=MODE, trace=False):
    nc = _get_nc(mode)
    in_maps = _prep_inputs(inputs)
    res = bass_utils.run_bass_kernel_spmd(
        nc, in_maps, core_ids=list(range(NCORES)), trace=trace)

    masks = np.asarray(inputs["masks"], np.float32)
    query = np.asarray(inputs["query"], np.float32)
    out = np.empty((B, S, H), np.float32)
    for c in range(NCORES):
        b, g = c // GROUPS, c % GROUPS
        hid = res.results[c]["hid"].reshape(HL, DH + 1, S)
        hT = hid[:, :DH, :]                      # (HL, DH, S)  (32x scaled)
        se = hid[:, DH, :]                       # (HL, S)
        blk = (hT / (ESC * se[:, None, :])).transpose(2, 0, 1).reshape(S, GH)
        out[b, :, g * GH:(g + 1) * GH] = blk
    out = out * masks[:, :, None] + query
    return out, res


def kernel(**inputs) -> np.ndarray:
    out, _ = run(inputs)
    return out
